# revision 13
# baseline (speedup 1.0000x reference)
"""Multi-head attention (B=2, S=2048, D=1024, H=16) on 8 Trainium2 NeuronCores.

Sharding: core c = (batch b = c//4) x (head-group g = c%4, 4 heads each).
Each core computes its 4 heads' attention for its batch plus the partial
output projection over its 256 W_o columns; the host sums the 4 group
partials per batch.

All matmuls fp16 (PSUM fp32).  Structure: 8 HALF-strands (qt, hp, j),
each covering 512 q columns for 2 heads.  Per kc one [128,1024] PSUM
pair-tile holds both heads' scoresT side by side (head A cols 0:512 from
PE row-group 0-63, head B cols 512:1024 from rows 64-127) so the two
score matmuls co-execute on distinct PE row tiles (Δstart ~4ns), and ONE
exp activation consumes both — the next kc's pair then unblocks as a
unit and fuses again.  Pair tiles are double-buffered (4 PSUM banks),
accs 1 bank/head ×4 (two half-strands overlap at boundaries) = 8 banks.

PV lhsT blocks are zero-padded to 128 weight columns [V(64)|ones|0*63]
so LDWEIGHTS takes the 4x fast-weight-load path; the ones column
accumulates softmax denominators in PSUM row 64.

The exp stream paces the steady state at ~1.07us/kc; PE slack per kc
(~0.3-0.4us) is filled with projection units placed just-in-time.  Each
half-strand's last two PVs + normalize defer into the next half-strand's
first iterations.  Output projection s4 units 0-7 run inside the last
two half-strands, 8-15 in the tail; output is fp16 (halves the out DMA).
"""

import sys

for _p in ("/opt/trn_rl_repo", "/root/.axon_site/_ro/trn_rl_repo"):
    if _p not in sys.path:
        sys.path.insert(0, _p)

import numpy as np

import concourse.mybir as mybir
import concourse.tile as tile
from concourse import bacc
from concourse.bass_utils import run_bass_kernel_spmd

F32 = mybir.dt.float32
F16 = mybir.dt.float16

B, S, D = 2, 2048, 1024
H, DK = 16, 64
HPC = 4          # heads per core
NCORES = 8
DC = 8           # number of 128-row chunks of D (contraction tiles)
SC = 4           # S chunks of 512 for the projections
KC = S // 128    # 16 k-chunks
VW = 128         # padded V-block width per (kc, head): [V(64) | ones | zeros]

_CACHED_NC = None


def _build_nc():
    nc = bacc.Bacc("TRN2", target_bir_lowering=False, debug=False)

    xs = nc.dram_tensor("xs", [SC, 128, DC * 512], F16, kind="ExternalInput")
    wq = nc.dram_tensor("wq", [128, DC * 256], F16, kind="ExternalInput")
    wk = nc.dram_tensor("wk", [128, DC * 256], F16, kind="ExternalInput")
    wv = nc.dram_tensor("wv", [128, DC * 256], F16, kind="ExternalInput")
    wo = nc.dram_tensor("wo", [2, 128, D], F16, kind="ExternalInput")
    out = nc.dram_tensor("out", [S, D], F16, kind="ExternalOutput")

    with tile.TileContext(nc) as tc:
        with (
            tc.tile_pool(name="persist", bufs=1) as pp,
            tc.tile_pool(name="ps_mm", bufs=2, space="PSUM") as ps_mm,
            tc.tile_pool(name="ps_acc", bufs=3, space="PSUM") as ps_acc,
            tc.tile_pool(name="ps_fill", bufs=1, space="PSUM") as ps_fill,
            tc.tile_pool(name="exp_pool", bufs=8) as ep,
            tc.tile_pool(name="out_pool", bufs=4) as op_,
            tc.tile_pool(name="nrm_pool", bufs=6) as np_,
        ):
            wk_sb = pp.tile([128, DC * 256], F16, tag="wk")
            wq_sb = pp.tile([128, DC * 256], F16, tag="wq")
            wv_sb = pp.tile([128, DC * 256], F16, tag="wv")
            x_sb = [
                pp.tile([128, DC * 512], F16, tag=f"x{i}", name=f"x_sb{i}")
                for i in range(SC)
            ]
            wo_sb = [
                pp.tile([128, D], F16, tag=f"wo{i}", name=f"wo_sb{i}")
                for i in range(2)
            ]

            # --- PE warm-up: the HAM clock gate holds the PE at 1.2GHz
            # until it sees ~3.4us of sustained busy.  Junk matmuls on the
            # ones tile (no DMA deps) warm it while the first transfers
            # land, and fill the DMA-wait gaps in the prologue chains so
            # the gate never re-throttles.
            ones_sb = pp.tile([128, 16], F32, tag="ones")
            nc.gpsimd.memset(ones_sb[:], 1.0)
            junk_ps = ps_fill.tile([16, 16], F32, tag="fl", name="ps_junk")

            def junk(n):
                for _ in range(n):
                    nc.tensor.matmul(
                        junk_ps[:], ones_sb[:, 0:16], ones_sb[:, 0:16],
                        start=True, stop=True,
                    )

            junk(30)

            # --- DMA head.  Measured queue facts: sync/scalar HWDGE
            # ~100GB/s each, gpsimd SWDGE ~50GB/s, ring depth ~4 (issues
            # throttle to transfer pace), and a DMA issue on the scalar
            # engine blocks the exp stream behind it — so all scalar issues
            # happen before the first exp.  x0 split so the prologue's
            # critical 1MB lands by ~15us; the junk block keeps the PE busy
            # (HAM warm) until then.
            for d in range(6):
                nc.sync.dma_start(
                    x_sb[0][:, d * 512 : (d + 1) * 512],
                    xs.ap()[0][:, d * 512 : (d + 1) * 512],
                )
            for d in (6, 7):
                nc.gpsimd.dma_start(
                    x_sb[0][:, d * 512 : (d + 1) * 512],
                    xs.ap()[0][:, d * 512 : (d + 1) * 512],
                )
            for d in range(DC):
                nc.scalar.dma_start(
                    wk_sb[:, d * 256 : (d + 1) * 256],
                    wk.ap()[:, d * 256 : (d + 1) * 256],
                )
            for d in range(DC):
                nc.scalar.dma_start(
                    wq_sb[:, d * 256 : (d + 1) * 256],
                    wq.ap()[:, d * 256 : (d + 1) * 256],
                )
            for d in range(0, DC, 2):
                nc.gpsimd.dma_start(
                    wv_sb[:, d * 256 : (d + 2) * 256],
                    wv.ap()[:, d * 256 : (d + 2) * 256],
                )
            for h in range(4):
                nc.scalar.dma_start(
                    x_sb[1][:, h * 1024 : (h + 1) * 1024],
                    xs.ap()[1][:, h * 1024 : (h + 1) * 1024],
                )
            for h in range(4):
                nc.sync.dma_start(
                    x_sb[2][:, h * 1024 : (h + 1) * 1024],
                    xs.ap()[2][:, h * 1024 : (h + 1) * 1024],
                )
            for h in range(4):
                nc.sync.dma_start(
                    x_sb[3][:, h * 1024 : (h + 1) * 1024],
                    xs.ap()[3][:, h * 1024 : (h + 1) * 1024],
                )
            for i in range(2):
                nc.gpsimd.dma_start(wo_sb[i][:], wo.ap()[i])

            qt_sb = [
                pp.tile([128, S], F16, tag=f"qt{i}", name=f"qt_sb{i}")
                for i in range(2)
            ]
            kt_sb = [
                pp.tile([128, S], F16, tag=f"kt{i}", name=f"kt_sb{i}")
                for i in range(2)
            ]
            vp_sb = pp.tile([128, KC * HPC * VW], F16, tag="vp")
            ot_sb = [
                pp.tile([128, S], F16, tag=f"ot{i}", name=f"ot_sb{i}")
                for i in range(2)
            ]

            # vp: per (kc, h) a 128-col block [V(64) | ones | 0*63]; zero the
            # pad + write ones via DVE in kc-order quarters so the first
            # v-copies aren't gated on the whole memset.
            for qtr in range(4):
                nc.vector.memset(vp_sb[:, qtr * 2048 : (qtr + 1) * 2048], 0.0)
                ones_ap = vp_sb[:, qtr * 2048 : (qtr + 1) * 2048].rearrange(
                    "p (b c) -> p b c", c=VW
                )[:, :, 64:65]
                nc.vector.tensor_copy(ones_ap, ones_sb[:].unsqueeze(-1))

            # ---- projection work units, split into ~0.4us STEPS so the
            # PE's in-order queue never holds a long filler chain in front
            # of the next kc's score pair (fillers own a dedicated 1-bank
            # PSUM slot and may span kc slots freely; PSUM accumulation
            # groups interleave fine across banks). ----
            def qk_steps(w_sb, t_sb, hp, sc):
                st = {}

                def mk(d0, d1, last):
                    def run():
                        if d0 == 0:
                            st["ps"] = ps_fill.tile(
                                [128, 512], F32, tag="fl", name="ps_qk"
                            )
                        for d in range(d0, d1):
                            nc.tensor.matmul(
                                st["ps"][:],
                                w_sb[:, d * 256 + hp * 128 : d * 256 + hp * 128 + 128],
                                x_sb[sc][:, d * 512 : (d + 1) * 512],
                                start=(d == 0),
                                stop=(d == DC - 1),
                            )
                        if last:
                            nc.vector.tensor_copy(
                                t_sb[hp][:, sc * 512 : (sc + 1) * 512], st["ps"][:]
                            )
                    return run

                return [mk(0, 3, False), mk(3, 6, False), mk(6, 8, True)]

            def v_steps(kc):
                sc, i = divmod(kc, 4)
                st = {}

                def mk(d0, d1, last):
                    def run():
                        if d0 == 0:
                            st["ps"] = ps_fill.tile(
                                [128, 512], F32, tag="fl", name="ps_v"
                            )
                        for d in range(d0, d1):
                            nc.tensor.matmul(
                                st["ps"][:, 0:256],
                                x_sb[sc][:, d * 512 + i * 128 : d * 512 + i * 128 + 128],
                                wv_sb[:, d * 256 : (d + 1) * 256],
                                start=(d == 0),
                                stop=(d == DC - 1),
                            )
                        if last:
                            base = kc * HPC * VW
                            dst = vp_sb[:, base : base + HPC * VW]
                            dst = dst.rearrange("p (g c) -> p g c", c=VW)[:, :, 0:DK]
                            src = st["ps"][:, 0:256].rearrange("p (g c) -> p g c", c=DK)
                            nc.vector.tensor_copy(dst, src)
                    return run

                return [mk(0, 4, False), mk(4, 8, True)]

            s4_tail = [False]

            def s4_steps(q16):
                st = {}

                def mk(dc2):
                    def run():
                        o_key = "o"
                        if dc2 == 0:
                            st[o_key] = op_.tile([128, D], F16, tag="o", name="o_sb")
                        o_sb = st[o_key]
                        ps = ps_fill.tile([128, 512], F32, tag="fl", name="ps_s4")
                        for hp in range(2):
                            nc.tensor.matmul(
                                ps[:],
                                ot_sb[hp][:, q16 * 128 : (q16 + 1) * 128],
                                wo_sb[hp][:, dc2 * 512 : (dc2 + 1) * 512],
                                start=(hp == 0),
                                stop=(hp == 1),
                            )
                        if s4_tail[0] and dc2 == 0:
                            nc.scalar.copy(
                                o_sb[:, dc2 * 512 : (dc2 + 1) * 512], ps[:]
                            )
                        else:
                            nc.vector.tensor_copy(
                                o_sb[:, dc2 * 512 : (dc2 + 1) * 512], ps[:]
                            )
                        if dc2 == 1:
                            if s4_tail[0]:
                                for h in range(2):
                                    nc.sync.dma_start(
                                        out.ap()[
                                            q16 * 128 : (q16 + 1) * 128,
                                            h * 512 : (h + 1) * 512,
                                        ],
                                        o_sb[:, h * 512 : (h + 1) * 512],
                                    )
                            else:
                                nc.sync.dma_start(
                                    out.ap()[q16 * 128 : (q16 + 1) * 128, :], o_sb[:]
                                )
                    return run

                return [mk(0), mk(1)]

            # ---- prologue: kt0.s0 and qt0.s0 interleaved per-d on the two
            # ps_mm slots (pair tiles aren't allocated yet), junk between so
            # DMA-wait gaps don't re-throttle the PE ----
            ps_kt = ps_mm.tile([128, 512], F32, tag="mm", name="ps_pkt")
            ps_qt = ps_mm.tile([128, 512], F32, tag="mm", name="ps_pqt")
            for d in range(DC):
                nc.tensor.matmul(
                    ps_kt[:],
                    wk_sb[:, d * 256 : d * 256 + 128],
                    x_sb[0][:, d * 512 : (d + 1) * 512],
                    start=(d == 0), stop=(d == DC - 1),
                )
                junk(6)
                nc.tensor.matmul(
                    ps_qt[:],
                    wq_sb[:, d * 256 : d * 256 + 128],
                    x_sb[0][:, d * 512 : (d + 1) * 512],
                    start=(d == 0), stop=(d == DC - 1),
                )
                junk(6)
            nc.vector.tensor_copy(kt_sb[0][:, 0:512], ps_kt[:])
            nc.vector.tensor_copy(qt_sb[0][:, 0:512], ps_qt[:])

            # ---- filler placement (si, kc) -> steps, just-in-time ----
            fillers = {}

            def place(si, kcs, steps):
                for kc, s in zip(kcs, steps):
                    fillers.setdefault((si, kc), []).append(s)

            vs = {k: v_steps(k) for k in range(KC)}
            s4s = [s4_steps(q) for q in range(16)]

            place(0, (1, 2, 3), qk_steps(wk_sb, kt_sb, 0, 1))
            place(0, (5, 6, 7), qk_steps(wk_sb, kt_sb, 0, 2))
            place(0, (9, 10, 11), qk_steps(wk_sb, kt_sb, 0, 3))
            place(0, (12, 13, 14), qk_steps(wq_sb, qt_sb, 0, 1))
            place(0, (0, 0), vs[0])
            place(0, (0, 1), vs[1])
            for k in range(2, KC):
                place(0, (k - 1, k), vs[k])
            place(1, (0, 1, 2), qk_steps(wq_sb, qt_sb, 0, 2))
            place(1, (8, 9, 10), qk_steps(wq_sb, qt_sb, 0, 3))
            place(2, (0, 1, 2), qk_steps(wk_sb, kt_sb, 1, 0))
            place(2, (6, 7, 8), qk_steps(wk_sb, kt_sb, 1, 1))
            place(2, (11, 12, 13), qk_steps(wq_sb, qt_sb, 1, 0))
            place(3, (0, 1, 2), qk_steps(wk_sb, kt_sb, 1, 2))
            place(3, (5, 6, 7), qk_steps(wk_sb, kt_sb, 1, 3))
            place(3, (10, 11, 12), qk_steps(wq_sb, qt_sb, 1, 1))
            place(4, (0, 1, 2), qk_steps(wq_sb, qt_sb, 1, 2))
            place(4, (8, 9, 10), qk_steps(wq_sb, qt_sb, 1, 3))
            place(5, (0, 1), s4s[0])
            place(5, (3, 4), s4s[1])
            place(5, (7, 8), s4s[2])
            place(5, (11, 12), s4s[3])
            place(6, (0, 1), s4s[4])
            place(6, (3, 4), s4s[5])
            place(6, (7, 8), s4s[6])
            place(6, (11, 12), s4s[7])
            place(7, (0, 1), s4s[8])
            place(7, (3, 4), s4s[9])
            place(7, (7, 8), s4s[10])
            place(7, (11, 12), s4s[11])
            # (qt, hp, j): hp-major, j innermost
            strands = [
                (0, 0, 0), (0, 0, 1), (1, 0, 0), (1, 0, 1),
                (0, 1, 0), (0, 1, 1), (1, 1, 0), (1, 1, 1),
            ]

            def normalize(accs, qt, hp, j, tail=False):
                dens, rs, rbs = {}, {}, {}
                for hsel in range(2):
                    den_sb = np_.tile([1, 512], F32, tag="den", name=f"den{hsel}")
                    if tail:
                        nc.scalar.copy(den_sb[:], accs[hsel][DK : DK + 1, :])
                    else:
                        nc.vector.tensor_copy(den_sb[:], accs[hsel][DK : DK + 1, :])
                    dens[hsel] = den_sb
                for hsel in range(2):
                    r_sb = np_.tile([1, 512], F32, tag="r", name=f"r{hsel}")
                    nc.vector.reciprocal_approx_fast(r_sb[:], dens[hsel][:])
                    rs[hsel] = r_sb
                for hsel in range(2):
                    rb_sb = np_.tile([64, 512], F32, tag="rb", name=f"rb{hsel}")
                    nc.gpsimd.partition_broadcast(rb_sb[:], rs[hsel][:])
                    rbs[hsel] = rb_sb
                q0 = qt * 1024 + j * 512
                for hsel in range(2):
                    nc.vector.tensor_mul(
                        ot_sb[hp][hsel * 64 : hsel * 64 + 64, q0 : q0 + 512],
                        accs[hsel][0:DK, :],
                        rbs[hsel][:],
                    )

            # Each half-strand's final PV + normalize defer into the NEXT
            # half-strand's first iteration (so its scores/exp launch first
            # and the exp stream never gaps at a boundary).
            pending = None

            for si, (qt, hp, j) in enumerate(strands):
                accs = {}

                def pv(kc, e_sb, accs=accs, hp=hp):
                    for hsel in range(2):
                        if hsel not in accs:
                            # lazy: the previous half-strand's accs live
                            # until its deferred normalize at our kc0
                            accs[hsel] = ps_acc.tile(
                                [128, 512], F32, tag="acc", name=f"acc{hsel}"
                            )
                        h = hp * 2 + hsel
                        nc.tensor.matmul(
                            accs[hsel][:],
                            vp_sb[:, (kc * HPC + h) * VW : (kc * HPC + h) * VW + VW],
                            e_sb[:, hsel * 512 : (hsel + 1) * 512],
                            start=(kc == 0),
                            stop=(kc == KC - 1),
                        )

                qoff = qt * 1024 + j * 512
                esq = []
                for kc in range(KC):
                    # one [128,1024] pair tile: head A scores in cols 0:512
                    # (PE rows 0-63), head B in 512:1024 (rows 64-127) —
                    # distinct row groups + banks, so the pair co-executes
                    pair = ps_mm.tile([128, 1024], F32, tag="mm", name="ps_sc")
                    for hsel in range(2):
                        p0 = hsel * 64
                        nc.tensor.matmul(
                            pair[:, hsel * 512 : (hsel + 1) * 512],
                            kt_sb[hp][p0 : p0 + 64, kc * 128 : (kc + 1) * 128],
                            qt_sb[hp][p0 : p0 + 64, qoff : qoff + 512],
                            start=True,
                            stop=True,
                        )
                    e_sb = ep.tile([128, 1024], F16, tag="e")
                    nc.scalar.activation(
                        e_sb[:], pair[:], mybir.ActivationFunctionType.Exp
                    )
                    if kc == 0 and pending is not None:
                        pending()
                        pending = None
                    for u in fillers.get((si, kc), ()):
                        u()
                    # PV trails the scores/exp stream by 2 kc
                    esq.append((kc, e_sb))
                    if len(esq) > 2:
                        pv(*esq.pop(0))

                pv(*esq.pop(0))
                if si < len(strands) - 1:
                    def mk_pending(pv=pv, args=esq[0], accs=accs, qt=qt, hp=hp, j=j):
                        def run():
                            pv(*args)
                            normalize(accs, qt, hp, j)
                        return run
                    pending = mk_pending()
                else:
                    jt = ps_fill.tile([16, 16], F32, tag="fl", name="ps_jt")
                    for _ in range(40):
                        nc.tensor.matmul(
                            jt[:], ones_sb[:, 0:16], ones_sb[:, 0:16],
                            start=True, stop=True,
                        )
                    pv(*esq[0])
                    for _ in range(30):
                        nc.tensor.matmul(
                            jt[:], ones_sb[:, 0:16], ones_sb[:, 0:16],
                            start=True, stop=True,
                        )
                    s4_tail[0] = True
                    dens, rs, rbs = {}, {}, {}
                    for hsel in range(2):
                        den_sb = np_.tile([1, 512], F32, tag="den", name=f"den{hsel}")
                        nc.scalar.copy(den_sb[:], accs[hsel][DK : DK + 1, :])
                        dens[hsel] = den_sb
                    for hsel in range(2):
                        r_sb = np_.tile([1, 512], F32, tag="r", name=f"r{hsel}")
                        nc.vector.reciprocal_approx_fast(r_sb[:], dens[hsel][:])
                        rs[hsel] = r_sb
                    for hsel in range(2):
                        rb_sb = np_.tile([64, 512], F32, tag="rb", name=f"rb{hsel}")
                        nc.gpsimd.partition_broadcast(rb_sb[:], rs[hsel][:])
                        rbs[hsel] = rb_sb
                    q0c = qt * 1024 + j * 512

                    def ot_mult_half(hsel, half):
                        c0 = q0c + half * 256
                        nc.vector.tensor_mul(
                            ot_sb[hp][hsel * 64 : hsel * 64 + 64, c0 : c0 + 256],
                            accs[hsel][0:DK, half * 256 : half * 256 + 256],
                            rbs[hsel][:, half * 256 : half * 256 + 256],
                        )

                    def s4_tail_unit(q, eng):
                        o_sb = op_.tile([128, D], F16, tag="o", name="o_sb")
                        ps = ps_mm.tile([128, D], F32, tag="mm", name="ps_s4t")
                        for dc2 in range(2):
                            for hp_ in range(2):
                                nc.tensor.matmul(
                                    ps[:, dc2 * 512 : (dc2 + 1) * 512],
                                    ot_sb[hp_][:, q * 128 : (q + 1) * 128],
                                    wo_sb[hp_][:, dc2 * 512 : (dc2 + 1) * 512],
                                    start=(hp_ == 0), stop=(hp_ == 1),
                                )
                        nc.scalar.copy(o_sb[:, 0:512], ps[:, 0:512])
                        nc.vector.tensor_copy(o_sb[:, 512:1024], ps[:, 512:1024])
                        for h in range(2):
                            eng.dma_start(
                                out.ap()[q * 128 : (q + 1) * 128, h * 512 : (h + 1) * 512],
                                o_sb[:, h * 512 : (h + 1) * 512],
                            )

                    ot_mult_half(0, 0)
                    ot_mult_half(1, 0)
                    s4_tail_unit(12, nc.sync)
                    s4_tail_unit(13, nc.scalar)
                    ot_mult_half(0, 1)
                    ot_mult_half(1, 1)
                    s4_tail_unit(14, nc.sync)
                    s4_tail_unit(15, nc.scalar)

    nc.compile()
    return nc
def _shard_inputs(x, W_q, W_k, W_v, W_o):
    """Build the 8 per-core input maps (fp16, C-contiguous)."""

    def pack_w(w_rows):  # [256, D] weight rows -> [128, DC*256] lhsT tiles
        wt = w_rows.T.astype(np.float16)  # [D, 256]
        return np.ascontiguousarray(
            wt.reshape(DC, 128, 256).transpose(1, 0, 2).reshape(128, DC * 256)
        )

    in_maps = []
    for c in range(NCORES):
        b, g = divmod(c, HPC)
        rows = slice(g * HPC * DK, (g + 1) * HPC * DK)
        xt = x[b].T.astype(np.float16)  # [D, S]
        xs = np.ascontiguousarray(
            xt.reshape(DC, 128, SC, 512).transpose(2, 1, 0, 3).reshape(SC, 128, DC * 512)
        )
        in_maps.append(
            {
                "xs": xs,
                "wq": pack_w(W_q[rows] * 0.125),
                "wk": pack_w(W_k[rows]),
                "wv": pack_w(W_v[rows]),
                "wo": np.ascontiguousarray(
                    W_o[:, rows].T.astype(np.float16).reshape(2, 128, D)
                ),
            }
        )
    return in_maps


def _numpy_fallback(x, attention_mask, W_q, W_k, W_v, W_o):
    """Exact reference path (only used if the mask is not all ones)."""
    out = np.empty((B, S, D), np.float32)
    for b in range(B):
        q = (x[b] @ W_q.T).reshape(S, H, DK).transpose(1, 0, 2)
        k = (x[b] @ W_k.T).reshape(S, H, DK).transpose(1, 0, 2)
        v = (x[b] @ W_v.T).reshape(S, H, DK).transpose(1, 0, 2)
        scores = np.einsum("hqd,hkd->hqk", q, k)
        scores = np.where(attention_mask[b][None, None, :] == 0, -np.inf, scores)
        scores = scores / np.sqrt(DK)
        scores -= scores.max(axis=-1, keepdims=True)
        w = np.exp(scores)
        w /= w.sum(axis=-1, keepdims=True)
        o = np.einsum("hqk,hkd->hqd", w, v).transpose(1, 0, 2).reshape(S, D)
        out[b] = o @ W_o.T
    return out


def kernel(x, attention_mask, W_q, W_k, W_v, W_o, _trace=False):
    global _CACHED_NC
    x = np.asarray(x, dtype=np.float32)
    attention_mask = np.asarray(attention_mask)
    W_q = np.asarray(W_q, dtype=np.float32)
    W_k = np.asarray(W_k, dtype=np.float32)
    W_v = np.asarray(W_v, dtype=np.float32)
    W_o = np.asarray(W_o, dtype=np.float32)

    if not np.all(attention_mask == 1):
        return _numpy_fallback(x, attention_mask, W_q, W_k, W_v, W_o)

    if _CACHED_NC is None:
        _CACHED_NC = _build_nc()
    nc = _CACHED_NC

    in_maps = _shard_inputs(x, W_q, W_k, W_v, W_o)
    res = run_bass_kernel_spmd(
        nc, in_maps, core_ids=list(range(NCORES)), trace=_trace
    )

    out = np.empty((B, S, D), np.float32)
    for b in range(B):
        acc = np.zeros((S, D), np.float64)
        for g in range(HPC):
            acc += res.results[b * HPC + g]["out"].astype(np.float64)
        out[b] = acc.astype(np.float32)
    if _trace:
        kernel.last_exec_time_ns = res.exec_time_ns
    return out


# revision 14
# speedup vs baseline: 1.0292x; 1.0292x over previous
"""Multi-head attention (B=2, S=2048, D=1024, H=16) on 8 Trainium2 NeuronCores.

Sharding: core c = (batch b = c//4) x (head-group g = c%4, 4 heads each).
Each core computes its 4 heads' attention for its batch plus the partial
output projection over its 256 W_o columns; the host sums the 4 group
partials per batch.

All matmuls fp16 (PSUM fp32).  Structure: 8 HALF-strands (qt, hp, j),
each covering 512 q columns for 2 heads.  Per kc one [128,1024] PSUM
pair-tile holds both heads' scoresT side by side (head A cols 0:512 from
PE row-group 0-63, head B cols 512:1024 from rows 64-127) so the two
score matmuls co-execute on distinct PE row tiles (Δstart ~4ns), and ONE
exp activation consumes both — the next kc's pair then unblocks as a
unit and fuses again.  Pair tiles are double-buffered (4 PSUM banks),
accs 1 bank/head ×4 (two half-strands overlap at boundaries) = 8 banks.

PV lhsT blocks are zero-padded to 128 weight columns [V(64)|ones|0*63]
so LDWEIGHTS takes the 4x fast-weight-load path; the ones column
accumulates softmax denominators in PSUM row 64.

The exp stream paces the steady state at ~1.07us/kc; PE slack per kc
(~0.3-0.4us) is filled with projection units placed just-in-time.  Each
half-strand's last two PVs + normalize defer into the next half-strand's
first iterations.  Output projection s4 units 0-7 run inside the last
two half-strands, 8-15 in the tail; output is fp16 (halves the out DMA).
"""

import sys

for _p in ("/opt/trn_rl_repo", "/root/.axon_site/_ro/trn_rl_repo"):
    if _p not in sys.path:
        sys.path.insert(0, _p)

import numpy as np

import concourse.mybir as mybir
import concourse.tile as tile
from concourse import bacc
from concourse.bass_utils import run_bass_kernel_spmd

F32 = mybir.dt.float32
F16 = mybir.dt.float16

B, S, D = 2, 2048, 1024
H, DK = 16, 64
HPC = 4          # heads per core
NCORES = 8
DC = 8           # number of 128-row chunks of D (contraction tiles)
SC = 4           # S chunks of 512 for the projections
KC = S // 128    # 16 k-chunks
VW = 128         # padded V-block width per (kc, head): [V(64) | ones | zeros]

_CACHED_NC = None


def _build_nc():
    nc = bacc.Bacc("TRN2", target_bir_lowering=False, debug=False)

    xs = nc.dram_tensor("xs", [SC, 128, DC * 512], F16, kind="ExternalInput")
    wq = nc.dram_tensor("wq", [128, DC * 256], F16, kind="ExternalInput")
    wk = nc.dram_tensor("wk", [128, DC * 256], F16, kind="ExternalInput")
    wv = nc.dram_tensor("wv", [128, DC * 256], F16, kind="ExternalInput")
    wo = nc.dram_tensor("wo", [2, 128, D], F16, kind="ExternalInput")
    out = nc.dram_tensor("out", [S, D], F16, kind="ExternalOutput")

    with tile.TileContext(nc) as tc:
        with (
            tc.tile_pool(name="persist", bufs=1) as pp,
            tc.tile_pool(name="ps_mm", bufs=2, space="PSUM") as ps_mm,
            tc.tile_pool(name="ps_acc", bufs=3, space="PSUM") as ps_acc,
            tc.tile_pool(name="ps_fill", bufs=1, space="PSUM") as ps_fill,
            tc.tile_pool(name="exp_pool", bufs=8) as ep,
            tc.tile_pool(name="out_pool", bufs=4) as op_,
            tc.tile_pool(name="nrm_pool", bufs=6) as np_,
        ):
            wk_sb = pp.tile([128, DC * 256], F16, tag="wk")
            wq_sb = pp.tile([128, DC * 256], F16, tag="wq")
            wv_sb = pp.tile([128, DC * 256], F16, tag="wv")
            x_sb = [
                pp.tile([128, DC * 512], F16, tag=f"x{i}", name=f"x_sb{i}")
                for i in range(SC)
            ]
            wo_sb = [
                pp.tile([128, D], F16, tag=f"wo{i}", name=f"wo_sb{i}")
                for i in range(2)
            ]

            # --- PE warm-up: the HAM clock gate holds the PE at 1.2GHz
            # until it sees ~3.4us of sustained busy.  Junk matmuls on the
            # ones tile (no DMA deps) warm it while the first transfers
            # land, and fill the DMA-wait gaps in the prologue chains so
            # the gate never re-throttles.
            ones_sb = pp.tile([128, 16], F32, tag="ones")
            nc.gpsimd.memset(ones_sb[:], 1.0)
            junk_ps = ps_fill.tile([16, 16], F32, tag="fl", name="ps_junk")

            def junk(n):
                for _ in range(n):
                    nc.tensor.matmul(
                        junk_ps[:], ones_sb[:, 0:16], ones_sb[:, 0:16],
                        start=True, stop=True,
                    )

            junk(80)

            # --- DMA head: chunked transfers; issue cost is ~0.6us per
            # dma_start and serializes per engine, so spread issues over the
            # three DGE-capable engines, first-needed first.
            # kt0.s0 consumes (wk.d, x0.d) pairs in d order; qt0.s0 needs wq.
            for d in range(0, DC, 2):
                nc.sync.dma_start(
                    wk_sb[:, d * 256 : (d + 1) * 256],
                    wk.ap()[:, d * 256 : (d + 1) * 256],
                )
                nc.gpsimd.dma_start(
                    x_sb[0][:, d * 512 : (d + 1) * 512],
                    xs.ap()[0][:, d * 512 : (d + 1) * 512],
                )
                nc.sync.dma_start(
                    x_sb[0][:, (d + 1) * 512 : (d + 2) * 512],
                    xs.ap()[0][:, (d + 1) * 512 : (d + 2) * 512],
                )
                nc.gpsimd.dma_start(
                    wk_sb[:, (d + 1) * 256 : (d + 2) * 256],
                    wk.ap()[:, (d + 1) * 256 : (d + 2) * 256],
                )
            for d in range(DC):
                nc.scalar.dma_start(
                    wq_sb[:, d * 256 : (d + 1) * 256],
                    wq.ap()[:, d * 256 : (d + 1) * 256],
                )
            # x1 split between sync and gpsimd (needed by kt0.s1 early in
            # half-strand 0); wv on gpsimd (v units start ~kc2)
            for h in range(2):
                nc.sync.dma_start(
                    x_sb[1][:, h * 2048 : h * 2048 + 1024],
                    xs.ap()[1][:, h * 2048 : h * 2048 + 1024],
                )
                nc.gpsimd.dma_start(
                    x_sb[1][:, h * 2048 + 1024 : (h + 1) * 2048],
                    xs.ap()[1][:, h * 2048 + 1024 : (h + 1) * 2048],
                )
            for d in range(0, DC, 2):
                nc.gpsimd.dma_start(
                    wv_sb[:, d * 256 : (d + 2) * 256],
                    wv.ap()[:, d * 256 : (d + 2) * 256],
                )
            for h in range(4):
                nc.sync.dma_start(
                    x_sb[2][:, h * 1024 : (h + 1) * 1024],
                    xs.ap()[2][:, h * 1024 : (h + 1) * 1024],
                )
                nc.scalar.dma_start(
                    x_sb[3][:, h * 1024 : (h + 1) * 1024],
                    xs.ap()[3][:, h * 1024 : (h + 1) * 1024],
                )
            for i in range(2):
                nc.scalar.dma_start(wo_sb[i][:], wo.ap()[i])

            qt_sb = [
                pp.tile([128, S], F16, tag=f"qt{i}", name=f"qt_sb{i}")
                for i in range(2)
            ]
            kt_sb = [
                pp.tile([128, S], F16, tag=f"kt{i}", name=f"kt_sb{i}")
                for i in range(2)
            ]
            vp_sb = pp.tile([128, KC * HPC * VW], F16, tag="vp")
            ot_sb = [
                pp.tile([128, S], F16, tag=f"ot{i}", name=f"ot_sb{i}")
                for i in range(2)
            ]

            # vp: per (kc, h) a 128-col block [V(64) | ones | 0*63]; zero the
            # pad + write ones via DVE in kc-order quarters so the first
            # v-copies aren't gated on the whole memset.
            for qtr in range(4):
                nc.vector.memset(vp_sb[:, qtr * 2048 : (qtr + 1) * 2048], 0.0)
                ones_ap = vp_sb[:, qtr * 2048 : (qtr + 1) * 2048].rearrange(
                    "p (b c) -> p b c", c=VW
                )[:, :, 64:65]
                nc.vector.tensor_copy(ones_ap, ones_sb[:].unsqueeze(-1))

            # ---- projection work units, split into ~0.4us STEPS so the
            # PE's in-order queue never holds a long filler chain in front
            # of the next kc's score pair (fillers own a dedicated 1-bank
            # PSUM slot and may span kc slots freely; PSUM accumulation
            # groups interleave fine across banks). ----
            def qk_steps(w_sb, t_sb, hp, sc):
                st = {}

                def mk(d0, d1, last):
                    def run():
                        if d0 == 0:
                            st["ps"] = ps_fill.tile(
                                [128, 512], F32, tag="fl", name="ps_qk"
                            )
                        for d in range(d0, d1):
                            nc.tensor.matmul(
                                st["ps"][:],
                                w_sb[:, d * 256 + hp * 128 : d * 256 + hp * 128 + 128],
                                x_sb[sc][:, d * 512 : (d + 1) * 512],
                                start=(d == 0),
                                stop=(d == DC - 1),
                            )
                        if last:
                            nc.vector.tensor_copy(
                                t_sb[hp][:, sc * 512 : (sc + 1) * 512], st["ps"][:]
                            )
                    return run

                return [mk(0, 3, False), mk(3, 6, False), mk(6, 8, True)]

            def v_steps(kc):
                sc, i = divmod(kc, 4)
                st = {}

                def mk(d0, d1, last):
                    def run():
                        if d0 == 0:
                            st["ps"] = ps_fill.tile(
                                [128, 512], F32, tag="fl", name="ps_v"
                            )
                        for d in range(d0, d1):
                            nc.tensor.matmul(
                                st["ps"][:, 0:256],
                                x_sb[sc][:, d * 512 + i * 128 : d * 512 + i * 128 + 128],
                                wv_sb[:, d * 256 : (d + 1) * 256],
                                start=(d == 0),
                                stop=(d == DC - 1),
                            )
                        if last:
                            base = kc * HPC * VW
                            dst = vp_sb[:, base : base + HPC * VW]
                            dst = dst.rearrange("p (g c) -> p g c", c=VW)[:, :, 0:DK]
                            src = st["ps"][:, 0:256].rearrange("p (g c) -> p g c", c=DK)
                            nc.vector.tensor_copy(dst, src)
                    return run

                return [mk(0, 4, False), mk(4, 8, True)]

            s4_tail = [False]

            def s4_steps(q16):
                st = {}

                def mk(dc2):
                    def run():
                        o_key = "o"
                        if dc2 == 0:
                            st[o_key] = op_.tile([128, D], F16, tag="o", name="o_sb")
                        o_sb = st[o_key]
                        ps = ps_fill.tile([128, 512], F32, tag="fl", name="ps_s4")
                        for hp in range(2):
                            nc.tensor.matmul(
                                ps[:],
                                ot_sb[hp][:, q16 * 128 : (q16 + 1) * 128],
                                wo_sb[hp][:, dc2 * 512 : (dc2 + 1) * 512],
                                start=(hp == 0),
                                stop=(hp == 1),
                            )
                        if s4_tail[0] and dc2 == 0:
                            nc.scalar.copy(
                                o_sb[:, dc2 * 512 : (dc2 + 1) * 512], ps[:]
                            )
                        else:
                            nc.vector.tensor_copy(
                                o_sb[:, dc2 * 512 : (dc2 + 1) * 512], ps[:]
                            )
                        if dc2 == 1:
                            if s4_tail[0]:
                                for h in range(2):
                                    nc.sync.dma_start(
                                        out.ap()[
                                            q16 * 128 : (q16 + 1) * 128,
                                            h * 512 : (h + 1) * 512,
                                        ],
                                        o_sb[:, h * 512 : (h + 1) * 512],
                                    )
                            else:
                                nc.sync.dma_start(
                                    out.ap()[q16 * 128 : (q16 + 1) * 128, :], o_sb[:]
                                )
                    return run

                return [mk(0), mk(1)]

            # ---- prologue: kt0.s0 and qt0.s0 interleaved per-d on the two
            # ps_mm slots (pair tiles aren't allocated yet), junk between so
            # DMA-wait gaps don't re-throttle the PE ----
            ps_kt = ps_mm.tile([128, 512], F32, tag="mm", name="ps_pkt")
            ps_qt = ps_mm.tile([128, 512], F32, tag="mm", name="ps_pqt")
            for d in range(DC):
                nc.tensor.matmul(
                    ps_kt[:],
                    wk_sb[:, d * 256 : d * 256 + 128],
                    x_sb[0][:, d * 512 : (d + 1) * 512],
                    start=(d == 0), stop=(d == DC - 1),
                )
                nc.tensor.matmul(
                    ps_qt[:],
                    wq_sb[:, d * 256 : d * 256 + 128],
                    x_sb[0][:, d * 512 : (d + 1) * 512],
                    start=(d == 0), stop=(d == DC - 1),
                )
            nc.vector.tensor_copy(kt_sb[0][:, 0:512], ps_kt[:])
            nc.vector.tensor_copy(qt_sb[0][:, 0:512], ps_qt[:])

            # ---- filler placement (si, kc) -> steps, just-in-time ----
            fillers = {}

            def place(si, kcs, steps):
                for kc, s in zip(kcs, steps):
                    fillers.setdefault((si, kc), []).append(s)

            vs = {k: v_steps(k) for k in range(KC)}
            s4s = [s4_steps(q) for q in range(16)]

            place(0, (1, 2, 3), qk_steps(wk_sb, kt_sb, 0, 1))
            place(0, (5, 6, 7), qk_steps(wk_sb, kt_sb, 0, 2))
            place(0, (9, 10, 11), qk_steps(wk_sb, kt_sb, 0, 3))
            place(0, (12, 13, 14), qk_steps(wq_sb, qt_sb, 0, 1))
            place(0, (0, 0), vs[0])
            place(0, (0, 1), vs[1])
            for k in range(2, KC):
                place(0, (k - 1, k), vs[k])
            place(1, (0, 1, 2), qk_steps(wq_sb, qt_sb, 0, 2))
            place(1, (8, 9, 10), qk_steps(wq_sb, qt_sb, 0, 3))
            place(2, (0, 1, 2), qk_steps(wk_sb, kt_sb, 1, 0))
            place(2, (6, 7, 8), qk_steps(wk_sb, kt_sb, 1, 1))
            place(2, (11, 12, 13), qk_steps(wq_sb, qt_sb, 1, 0))
            place(3, (0, 1, 2), qk_steps(wk_sb, kt_sb, 1, 2))
            place(3, (5, 6, 7), qk_steps(wk_sb, kt_sb, 1, 3))
            place(3, (10, 11, 12), qk_steps(wq_sb, qt_sb, 1, 1))
            place(4, (0, 1, 2), qk_steps(wq_sb, qt_sb, 1, 2))
            place(4, (8, 9, 10), qk_steps(wq_sb, qt_sb, 1, 3))
            place(5, (0, 1), s4s[0])
            place(5, (3, 4), s4s[1])
            place(5, (7, 8), s4s[2])
            place(5, (11, 12), s4s[3])
            place(6, (0, 1), s4s[4])
            place(6, (3, 4), s4s[5])
            place(6, (7, 8), s4s[6])
            place(6, (11, 12), s4s[7])
            place(7, (0, 1), s4s[8])
            place(7, (3, 4), s4s[9])
            place(7, (7, 8), s4s[10])
            place(7, (11, 12), s4s[11])
            # (qt, hp, j): hp-major, j innermost
            strands = [
                (0, 0, 0), (0, 0, 1), (1, 0, 0), (1, 0, 1),
                (0, 1, 0), (0, 1, 1), (1, 1, 0), (1, 1, 1),
            ]

            def normalize(accs, qt, hp, j, tail=False):
                dens, rs, rbs = {}, {}, {}
                for hsel in range(2):
                    den_sb = np_.tile([1, 512], F32, tag="den", name=f"den{hsel}")
                    if tail:
                        nc.scalar.copy(den_sb[:], accs[hsel][DK : DK + 1, :])
                    else:
                        nc.vector.tensor_copy(den_sb[:], accs[hsel][DK : DK + 1, :])
                    dens[hsel] = den_sb
                for hsel in range(2):
                    r_sb = np_.tile([1, 512], F32, tag="r", name=f"r{hsel}")
                    nc.vector.reciprocal_approx_fast(r_sb[:], dens[hsel][:])
                    rs[hsel] = r_sb
                for hsel in range(2):
                    rb_sb = np_.tile([64, 512], F32, tag="rb", name=f"rb{hsel}")
                    nc.gpsimd.partition_broadcast(rb_sb[:], rs[hsel][:])
                    rbs[hsel] = rb_sb
                q0 = qt * 1024 + j * 512
                for hsel in range(2):
                    nc.vector.tensor_mul(
                        ot_sb[hp][hsel * 64 : hsel * 64 + 64, q0 : q0 + 512],
                        accs[hsel][0:DK, :],
                        rbs[hsel][:],
                    )

            # Each half-strand's final PV + normalize defer into the NEXT
            # half-strand's first iteration (so its scores/exp launch first
            # and the exp stream never gaps at a boundary).
            pending = None

            for si, (qt, hp, j) in enumerate(strands):
                accs = {}

                def pv(kc, e_sb, accs=accs, hp=hp):
                    for hsel in range(2):
                        if hsel not in accs:
                            # lazy: the previous half-strand's accs live
                            # until its deferred normalize at our kc0
                            accs[hsel] = ps_acc.tile(
                                [128, 512], F32, tag="acc", name=f"acc{hsel}"
                            )
                        h = hp * 2 + hsel
                        nc.tensor.matmul(
                            accs[hsel][:],
                            vp_sb[:, (kc * HPC + h) * VW : (kc * HPC + h) * VW + VW],
                            e_sb[:, hsel * 512 : (hsel + 1) * 512],
                            start=(kc == 0),
                            stop=(kc == KC - 1),
                        )

                qoff = qt * 1024 + j * 512
                esq = []
                for kc in range(KC):
                    # one [128,1024] pair tile: head A scores in cols 0:512
                    # (PE rows 0-63), head B in 512:1024 (rows 64-127) —
                    # distinct row groups + banks, so the pair co-executes
                    pair = ps_mm.tile([128, 1024], F32, tag="mm", name="ps_sc")
                    for hsel in range(2):
                        p0 = hsel * 64
                        nc.tensor.matmul(
                            pair[:, hsel * 512 : (hsel + 1) * 512],
                            kt_sb[hp][p0 : p0 + 64, kc * 128 : (kc + 1) * 128],
                            qt_sb[hp][p0 : p0 + 64, qoff : qoff + 512],
                            start=True,
                            stop=True,
                        )
                    e_sb = ep.tile([128, 1024], F16, tag="e")
                    nc.scalar.activation(
                        e_sb[:], pair[:], mybir.ActivationFunctionType.Exp
                    )
                    if kc == 0 and pending is not None:
                        pending()
                        pending = None
                    for u in fillers.get((si, kc), ()):
                        u()
                    # PV trails the scores/exp stream by 2 kc
                    esq.append((kc, e_sb))
                    if len(esq) > 2:
                        pv(*esq.pop(0))

                pv(*esq.pop(0))
                if si < len(strands) - 1:
                    def mk_pending(pv=pv, args=esq[0], accs=accs, qt=qt, hp=hp, j=j):
                        def run():
                            pv(*args)
                            normalize(accs, qt, hp, j)
                        return run
                    pending = mk_pending()
                else:
                    pv(*esq[0])
                    s4_tail[0] = True
                    dens, rs, rbs = {}, {}, {}
                    for hsel in range(2):
                        den_sb = np_.tile([1, 512], F32, tag="den", name=f"den{hsel}")
                        nc.scalar.copy(den_sb[:], accs[hsel][DK : DK + 1, :])
                        dens[hsel] = den_sb
                    for hsel in range(2):
                        r_sb = np_.tile([1, 512], F32, tag="r", name=f"r{hsel}")
                        nc.vector.reciprocal_approx_fast(r_sb[:], dens[hsel][:])
                        rs[hsel] = r_sb
                    for hsel in range(2):
                        rb_sb = np_.tile([64, 512], F32, tag="rb", name=f"rb{hsel}")
                        nc.gpsimd.partition_broadcast(rb_sb[:], rs[hsel][:])
                        rbs[hsel] = rb_sb
                    q0c = qt * 1024 + j * 512

                    def ot_mult_half(hsel, half):
                        c0 = q0c + half * 256
                        nc.vector.tensor_mul(
                            ot_sb[hp][hsel * 64 : hsel * 64 + 64, c0 : c0 + 256],
                            accs[hsel][0:DK, half * 256 : half * 256 + 256],
                            rbs[hsel][:, half * 256 : half * 256 + 256],
                        )

                    def s4_tail_unit(q, eng):
                        o_sb = op_.tile([128, D], F16, tag="o", name="o_sb")
                        ps = ps_mm.tile([128, D], F32, tag="mm", name="ps_s4t")
                        for dc2 in range(2):
                            for hp_ in range(2):
                                nc.tensor.matmul(
                                    ps[:, dc2 * 512 : (dc2 + 1) * 512],
                                    ot_sb[hp_][:, q * 128 : (q + 1) * 128],
                                    wo_sb[hp_][:, dc2 * 512 : (dc2 + 1) * 512],
                                    start=(hp_ == 0), stop=(hp_ == 1),
                                )
                        nc.scalar.copy(o_sb[:, 0:512], ps[:, 0:512])
                        nc.vector.tensor_copy(o_sb[:, 512:1024], ps[:, 512:1024])
                        for h in range(2):
                            eng.dma_start(
                                out.ap()[q * 128 : (q + 1) * 128, h * 512 : (h + 1) * 512],
                                o_sb[:, h * 512 : (h + 1) * 512],
                            )

                    ot_mult_half(0, 0)
                    ot_mult_half(1, 0)
                    s4_tail_unit(12, nc.sync)
                    s4_tail_unit(13, nc.scalar)
                    ot_mult_half(0, 1)
                    ot_mult_half(1, 1)
                    s4_tail_unit(14, nc.sync)
                    s4_tail_unit(15, nc.scalar)

    nc.compile()
    return nc
def _shard_inputs(x, W_q, W_k, W_v, W_o):
    """Build the 8 per-core input maps (fp16, C-contiguous)."""

    def pack_w(w_rows):  # [256, D] weight rows -> [128, DC*256] lhsT tiles
        wt = w_rows.T.astype(np.float16)  # [D, 256]
        return np.ascontiguousarray(
            wt.reshape(DC, 128, 256).transpose(1, 0, 2).reshape(128, DC * 256)
        )

    in_maps = []
    for c in range(NCORES):
        b, g = divmod(c, HPC)
        rows = slice(g * HPC * DK, (g + 1) * HPC * DK)
        xt = x[b].T.astype(np.float16)  # [D, S]
        xs = np.ascontiguousarray(
            xt.reshape(DC, 128, SC, 512).transpose(2, 1, 0, 3).reshape(SC, 128, DC * 512)
        )
        in_maps.append(
            {
                "xs": xs,
                "wq": pack_w(W_q[rows] * 0.125),
                "wk": pack_w(W_k[rows]),
                "wv": pack_w(W_v[rows]),
                "wo": np.ascontiguousarray(
                    W_o[:, rows].T.astype(np.float16).reshape(2, 128, D)
                ),
            }
        )
    return in_maps


def _numpy_fallback(x, attention_mask, W_q, W_k, W_v, W_o):
    """Exact reference path (only used if the mask is not all ones)."""
    out = np.empty((B, S, D), np.float32)
    for b in range(B):
        q = (x[b] @ W_q.T).reshape(S, H, DK).transpose(1, 0, 2)
        k = (x[b] @ W_k.T).reshape(S, H, DK).transpose(1, 0, 2)
        v = (x[b] @ W_v.T).reshape(S, H, DK).transpose(1, 0, 2)
        scores = np.einsum("hqd,hkd->hqk", q, k)
        scores = np.where(attention_mask[b][None, None, :] == 0, -np.inf, scores)
        scores = scores / np.sqrt(DK)
        scores -= scores.max(axis=-1, keepdims=True)
        w = np.exp(scores)
        w /= w.sum(axis=-1, keepdims=True)
        o = np.einsum("hqk,hkd->hqd", w, v).transpose(1, 0, 2).reshape(S, D)
        out[b] = o @ W_o.T
    return out


def kernel(x, attention_mask, W_q, W_k, W_v, W_o, _trace=False):
    global _CACHED_NC
    x = np.asarray(x, dtype=np.float32)
    attention_mask = np.asarray(attention_mask)
    W_q = np.asarray(W_q, dtype=np.float32)
    W_k = np.asarray(W_k, dtype=np.float32)
    W_v = np.asarray(W_v, dtype=np.float32)
    W_o = np.asarray(W_o, dtype=np.float32)

    if not np.all(attention_mask == 1):
        return _numpy_fallback(x, attention_mask, W_q, W_k, W_v, W_o)

    if _CACHED_NC is None:
        _CACHED_NC = _build_nc()
    nc = _CACHED_NC

    in_maps = _shard_inputs(x, W_q, W_k, W_v, W_o)
    res = run_bass_kernel_spmd(
        nc, in_maps, core_ids=list(range(NCORES)), trace=_trace
    )

    out = np.empty((B, S, D), np.float32)
    for b in range(B):
        acc = np.zeros((S, D), np.float64)
        for g in range(HPC):
            acc += res.results[b * HPC + g]["out"].astype(np.float64)
        out[b] = acc.astype(np.float32)
    if _trace:
        kernel.last_exec_time_ns = res.exec_time_ns
    return out


# revision 17
# speedup vs baseline: 1.0342x; 1.0048x over previous
"""Multi-head attention (B=2, S=2048, D=1024, H=16) on 8 Trainium2 NeuronCores.

Sharding: core c = (batch b = c//4) x (head-group g = c%4, 4 heads each).
Each core computes its 4 heads' attention for its batch plus the partial
output projection over its 256 W_o columns; the host sums the 4 group
partials per batch.

All matmuls fp16 (PSUM fp32).  Structure: 8 HALF-strands (qt, hp, j),
each covering 512 q columns for 2 heads.  Per kc one [128,1024] PSUM
pair-tile holds both heads' scoresT side by side (head A cols 0:512 from
PE row-group 0-63, head B cols 512:1024 from rows 64-127) so the two
score matmuls co-execute on distinct PE row tiles (Δstart ~4ns), and ONE
exp activation consumes both — the next kc's pair then unblocks as a
unit and fuses again.  Pair tiles are double-buffered (4 PSUM banks),
accs 1 bank/head ×4 (two half-strands overlap at boundaries) = 8 banks.

PV lhsT blocks are zero-padded to 128 weight columns [V(64)|ones|0*63]
so LDWEIGHTS takes the 4x fast-weight-load path; the ones column
accumulates softmax denominators in PSUM row 64.

The exp stream paces the steady state at ~1.07us/kc; PE slack per kc
(~0.3-0.4us) is filled with projection units placed just-in-time.  Each
half-strand's last two PVs + normalize defer into the next half-strand's
first iterations.  Output projection s4 units 0-7 run inside the last
two half-strands, 8-15 in the tail; output is fp16 (halves the out DMA).
"""

import sys

for _p in ("/opt/trn_rl_repo", "/root/.axon_site/_ro/trn_rl_repo"):
    if _p not in sys.path:
        sys.path.insert(0, _p)

import numpy as np

import concourse.mybir as mybir
import concourse.tile as tile
from concourse import bacc
from concourse.bass_utils import run_bass_kernel_spmd

F32 = mybir.dt.float32
F16 = mybir.dt.float16

B, S, D = 2, 2048, 1024
H, DK = 16, 64
HPC = 4          # heads per core
NCORES = 8
DC = 8           # number of 128-row chunks of D (contraction tiles)
SC = 4           # S chunks of 512 for the projections
KC = S // 128    # 16 k-chunks
VW = 128         # padded V-block width per (kc, head): [V(64) | ones | zeros]

_CACHED_NC = None


def _build_nc():
    nc = bacc.Bacc("TRN2", target_bir_lowering=False, debug=False)

    xs = nc.dram_tensor("xs", [SC, 128, DC * 512], F16, kind="ExternalInput")
    wq = nc.dram_tensor("wq", [128, DC * 256], F16, kind="ExternalInput")
    wk = nc.dram_tensor("wk", [128, DC * 256], F16, kind="ExternalInput")
    wv = nc.dram_tensor("wv", [128, DC * 256], F16, kind="ExternalInput")
    wo = nc.dram_tensor("wo", [2, 128, D], F16, kind="ExternalInput")
    out = nc.dram_tensor("out", [S, D], F16, kind="ExternalOutput")

    with tile.TileContext(nc) as tc:
        with (
            tc.tile_pool(name="persist", bufs=1) as pp,
            tc.tile_pool(name="ps_mm", bufs=2, space="PSUM") as ps_mm,
            tc.tile_pool(name="ps_acc", bufs=3, space="PSUM") as ps_acc,
            tc.tile_pool(name="ps_fill", bufs=1, space="PSUM") as ps_fill,
            tc.tile_pool(name="exp_pool", bufs=8) as ep,
            tc.tile_pool(name="out_pool", bufs=4) as op_,
            tc.tile_pool(name="nrm_pool", bufs=6) as np_,
        ):
            wk_sb = pp.tile([128, DC * 256], F16, tag="wk")
            wq_sb = pp.tile([128, DC * 256], F16, tag="wq")
            wv_sb = pp.tile([128, DC * 256], F16, tag="wv")
            x_sb = [
                pp.tile([128, DC * 512], F16, tag=f"x{i}", name=f"x_sb{i}")
                for i in range(SC)
            ]
            wo_sb = [
                pp.tile([128, D], F16, tag=f"wo{i}", name=f"wo_sb{i}")
                for i in range(2)
            ]

            # --- PE warm-up: the HAM clock gate holds the PE at 1.2GHz
            # until it sees ~3.4us of sustained busy.  Junk matmuls on the
            # ones tile (no DMA deps) warm it while the first transfers
            # land, and fill the DMA-wait gaps in the prologue chains so
            # the gate never re-throttles.
            ones_sb = pp.tile([128, 16], F32, tag="ones")
            nc.gpsimd.memset(ones_sb[:], 1.0)
            junk_ps = ps_fill.tile([16, 16], F32, tag="fl", name="ps_junk")

            def junk(n):
                for _ in range(n):
                    nc.tensor.matmul(
                        junk_ps[:], ones_sb[:, 0:16], ones_sb[:, 0:16],
                        start=True, stop=True,
                    )

            junk(80)

            # --- DMA head: chunked transfers; issue cost is ~0.6us per
            # dma_start and serializes per engine, so spread issues over the
            # three DGE-capable engines, first-needed first.
            # kt0.s0 consumes (wk.d, x0.d) pairs in d order; qt0.s0 needs wq.
            for d in range(0, DC, 2):
                nc.sync.dma_start(
                    wk_sb[:, d * 256 : (d + 1) * 256],
                    wk.ap()[:, d * 256 : (d + 1) * 256],
                )
                nc.gpsimd.dma_start(
                    x_sb[0][:, d * 512 : (d + 1) * 512],
                    xs.ap()[0][:, d * 512 : (d + 1) * 512],
                )
                nc.sync.dma_start(
                    x_sb[0][:, (d + 1) * 512 : (d + 2) * 512],
                    xs.ap()[0][:, (d + 1) * 512 : (d + 2) * 512],
                )
                nc.gpsimd.dma_start(
                    wk_sb[:, (d + 1) * 256 : (d + 2) * 256],
                    wk.ap()[:, (d + 1) * 256 : (d + 2) * 256],
                )
            for d in range(DC):
                nc.scalar.dma_start(
                    wq_sb[:, d * 256 : (d + 1) * 256],
                    wq.ap()[:, d * 256 : (d + 1) * 256],
                )
            # x1 split between sync and gpsimd (needed by kt0.s1 early in
            # half-strand 0); wv on gpsimd (v units start ~kc2)
            for h in range(2):
                nc.sync.dma_start(
                    x_sb[1][:, h * 2048 : h * 2048 + 1024],
                    xs.ap()[1][:, h * 2048 : h * 2048 + 1024],
                )
                nc.gpsimd.dma_start(
                    x_sb[1][:, h * 2048 + 1024 : (h + 1) * 2048],
                    xs.ap()[1][:, h * 2048 + 1024 : (h + 1) * 2048],
                )
            for d in range(0, DC, 2):
                nc.gpsimd.dma_start(
                    wv_sb[:, d * 256 : (d + 2) * 256],
                    wv.ap()[:, d * 256 : (d + 2) * 256],
                )
            for h in range(4):
                nc.sync.dma_start(
                    x_sb[2][:, h * 1024 : (h + 1) * 1024],
                    xs.ap()[2][:, h * 1024 : (h + 1) * 1024],
                )
                nc.scalar.dma_start(
                    x_sb[3][:, h * 1024 : (h + 1) * 1024],
                    xs.ap()[3][:, h * 1024 : (h + 1) * 1024],
                )
            for i in range(2):
                nc.scalar.dma_start(wo_sb[i][:], wo.ap()[i])

            qt_sb = [
                pp.tile([128, S], F16, tag=f"qt{i}", name=f"qt_sb{i}")
                for i in range(2)
            ]
            kt_sb = [
                pp.tile([128, S], F16, tag=f"kt{i}", name=f"kt_sb{i}")
                for i in range(2)
            ]
            vp_sb = pp.tile([128, KC * HPC * VW], F16, tag="vp")
            ot_sb = [
                pp.tile([128, S], F16, tag=f"ot{i}", name=f"ot_sb{i}")
                for i in range(2)
            ]

            # vp: per (kc, h) a 128-col block [V(64) | ones | 0*63]; zero the
            # pad + write ones via DVE in kc-order quarters so the first
            # v-copies aren't gated on the whole memset.
            for qtr in range(4):
                nc.vector.memset(vp_sb[:, qtr * 2048 : (qtr + 1) * 2048], 0.0)
                ones_ap = vp_sb[:, qtr * 2048 : (qtr + 1) * 2048].rearrange(
                    "p (b c) -> p b c", c=VW
                )[:, :, 64:65]
                nc.vector.tensor_copy(ones_ap, ones_sb[:].unsqueeze(-1))

            # ---- projection work units, split into ~0.4us STEPS so the
            # PE's in-order queue never holds a long filler chain in front
            # of the next kc's score pair (fillers own a dedicated 1-bank
            # PSUM slot and may span kc slots freely; PSUM accumulation
            # groups interleave fine across banks). ----
            def qk_steps(w_sb, t_sb, hp, sc):
                st = {}

                def mk(d0, d1, last):
                    def run():
                        if d0 == 0:
                            st["ps"] = ps_fill.tile(
                                [128, 512], F32, tag="fl", name="ps_qk"
                            )
                        for d in range(d0, d1):
                            nc.tensor.matmul(
                                st["ps"][:],
                                w_sb[:, d * 256 + hp * 128 : d * 256 + hp * 128 + 128],
                                x_sb[sc][:, d * 512 : (d + 1) * 512],
                                start=(d == 0),
                                stop=(d == DC - 1),
                            )
                        if last:
                            nc.vector.tensor_copy(
                                t_sb[hp][:, sc * 512 : (sc + 1) * 512], st["ps"][:]
                            )
                    return run

                return [mk(0, 3, False), mk(3, 6, False), mk(6, 8, True)]

            def v_steps(kc):
                sc, i = divmod(kc, 4)
                st = {}

                def mk(d0, d1, last):
                    def run():
                        if d0 == 0:
                            st["ps"] = ps_fill.tile(
                                [128, 512], F32, tag="fl", name="ps_v"
                            )
                        for d in range(d0, d1):
                            nc.tensor.matmul(
                                st["ps"][:, 0:256],
                                x_sb[sc][:, d * 512 + i * 128 : d * 512 + i * 128 + 128],
                                wv_sb[:, d * 256 : (d + 1) * 256],
                                start=(d == 0),
                                stop=(d == DC - 1),
                            )
                        if last:
                            base = kc * HPC * VW
                            dst = vp_sb[:, base : base + HPC * VW]
                            dst = dst.rearrange("p (g c) -> p g c", c=VW)[:, :, 0:DK]
                            src = st["ps"][:, 0:256].rearrange("p (g c) -> p g c", c=DK)
                            nc.vector.tensor_copy(dst, src)
                    return run

                return [mk(0, 4, False), mk(4, 8, True)]

            s4_tail = [False]

            def s4_steps(q16):
                st = {}

                def mk(dc2):
                    def run():
                        o_key = "o"
                        if dc2 == 0:
                            st[o_key] = op_.tile([128, D], F16, tag="o", name="o_sb")
                        o_sb = st[o_key]
                        ps = ps_fill.tile([128, 512], F32, tag="fl", name="ps_s4")
                        for hp in range(2):
                            nc.tensor.matmul(
                                ps[:],
                                ot_sb[hp][:, q16 * 128 : (q16 + 1) * 128],
                                wo_sb[hp][:, dc2 * 512 : (dc2 + 1) * 512],
                                start=(hp == 0),
                                stop=(hp == 1),
                            )
                        if s4_tail[0] and dc2 == 0:
                            nc.scalar.copy(
                                o_sb[:, dc2 * 512 : (dc2 + 1) * 512], ps[:]
                            )
                        else:
                            nc.vector.tensor_copy(
                                o_sb[:, dc2 * 512 : (dc2 + 1) * 512], ps[:]
                            )
                        if dc2 == 1:
                            if s4_tail[0]:
                                for h in range(2):
                                    nc.sync.dma_start(
                                        out.ap()[
                                            q16 * 128 : (q16 + 1) * 128,
                                            h * 512 : (h + 1) * 512,
                                        ],
                                        o_sb[:, h * 512 : (h + 1) * 512],
                                    )
                            else:
                                nc.sync.dma_start(
                                    out.ap()[q16 * 128 : (q16 + 1) * 128, :], o_sb[:]
                                )
                    return run

                return [mk(0), mk(1)]

            # ---- prologue: kt0.s0 and qt0.s0 interleaved per-d on the two
            # ps_mm slots (pair tiles aren't allocated yet), junk between so
            # DMA-wait gaps don't re-throttle the PE ----
            ps_kt = ps_mm.tile([128, 512], F32, tag="mm", name="ps_pkt")
            ps_qt = ps_mm.tile([128, 512], F32, tag="mm", name="ps_pqt")
            for d in range(DC):
                nc.tensor.matmul(
                    ps_kt[:],
                    wk_sb[:, d * 256 : d * 256 + 128],
                    x_sb[0][:, d * 512 : (d + 1) * 512],
                    start=(d == 0), stop=(d == DC - 1),
                )
                nc.tensor.matmul(
                    ps_qt[:],
                    wq_sb[:, d * 256 : d * 256 + 128],
                    x_sb[0][:, d * 512 : (d + 1) * 512],
                    start=(d == 0), stop=(d == DC - 1),
                )
            nc.vector.tensor_copy(kt_sb[0][:, 0:512], ps_kt[:])
            nc.vector.tensor_copy(qt_sb[0][:, 0:512], ps_qt[:])

            # ---- filler placement (si, kc) -> steps, just-in-time ----
            fillers = {}

            def place(si, kcs, steps):
                for kc, s in zip(kcs, steps):
                    fillers.setdefault((si, kc), []).append(s)

            vs = {k: v_steps(k) for k in range(KC)}
            s4s = [s4_steps(q) for q in range(16)]

            place(0, (1, 2, 3), qk_steps(wk_sb, kt_sb, 0, 1))
            place(0, (5, 6, 7), qk_steps(wk_sb, kt_sb, 0, 2))
            place(0, (9, 10, 11), qk_steps(wk_sb, kt_sb, 0, 3))
            place(0, (12, 13, 14), qk_steps(wq_sb, qt_sb, 0, 1))
            place(0, (0, 0), vs[0])
            place(0, (0, 1), vs[1])
            for k in range(2, 14):
                place(0, (k - 1, k), vs[k])
            place(1, (0, 1, 2), qk_steps(wq_sb, qt_sb, 0, 2))
            place(1, (8, 9, 10), qk_steps(wq_sb, qt_sb, 0, 3))
            place(2, (0, 1, 2), qk_steps(wk_sb, kt_sb, 1, 0))
            place(2, (6, 7, 8), qk_steps(wk_sb, kt_sb, 1, 1))
            place(2, (11, 12, 13), qk_steps(wq_sb, qt_sb, 1, 0))
            place(3, (0, 1, 2), qk_steps(wk_sb, kt_sb, 1, 2))
            place(3, (5, 6, 7), qk_steps(wk_sb, kt_sb, 1, 3))
            place(3, (10, 11, 12), qk_steps(wq_sb, qt_sb, 1, 1))
            place(4, (0, 1, 2), qk_steps(wq_sb, qt_sb, 1, 2))
            place(4, (8, 9, 10), qk_steps(wq_sb, qt_sb, 1, 3))
            place(5, (0, 1), s4s[0])
            place(5, (3, 4), s4s[1])
            place(5, (7, 8), s4s[2])
            place(5, (11, 12), s4s[3])
            place(6, (0, 1), s4s[4])
            place(6, (3, 4), s4s[5])
            place(6, (7, 8), s4s[6])
            place(6, (11, 12), s4s[7])
            place(7, (0, 1), s4s[8])
            place(7, (3, 4), s4s[9])
            place(7, (7, 8), s4s[10])
            place(7, (11, 12), s4s[11])
            # (qt, hp, j): hp-major, j innermost
            strands = [
                (0, 0, 0), (0, 0, 1), (1, 0, 0), (1, 0, 1),
                (0, 1, 0), (0, 1, 1), (1, 1, 0), (1, 1, 1),
            ]

            def normalize(accs, qt, hp, j, tail=False):
                dens, rs, rbs = {}, {}, {}
                for hsel in range(2):
                    den_sb = np_.tile([1, 512], F32, tag="den", name=f"den{hsel}")
                    if tail:
                        nc.scalar.copy(den_sb[:], accs[hsel][DK : DK + 1, :])
                    else:
                        nc.vector.tensor_copy(den_sb[:], accs[hsel][DK : DK + 1, :])
                    dens[hsel] = den_sb
                for hsel in range(2):
                    r_sb = np_.tile([1, 512], F32, tag="r", name=f"r{hsel}")
                    nc.vector.reciprocal_approx_fast(r_sb[:], dens[hsel][:])
                    rs[hsel] = r_sb
                for hsel in range(2):
                    rb_sb = np_.tile([64, 512], F32, tag="rb", name=f"rb{hsel}")
                    nc.gpsimd.partition_broadcast(rb_sb[:], rs[hsel][:])
                    rbs[hsel] = rb_sb
                q0 = qt * 1024 + j * 512
                for hsel in range(2):
                    nc.vector.tensor_mul(
                        ot_sb[hp][hsel * 64 : hsel * 64 + 64, q0 : q0 + 512],
                        accs[hsel][0:DK, :],
                        rbs[hsel][:],
                    )

            # Each half-strand's final PV + normalize defer into the NEXT
            # half-strand's first iteration (so its scores/exp launch first
            # and the exp stream never gaps at a boundary).
            pending = []

            for si, (qt, hp, j) in enumerate(strands):
                accs = {}

                def pv(kc, e_sb, accs=accs, hp=hp):
                    for hsel in range(2):
                        if hsel not in accs:
                            # lazy: the previous half-strand's accs live
                            # until its deferred normalize at our kc0
                            accs[hsel] = ps_acc.tile(
                                [128, 512], F32, tag="acc", name=f"acc{hsel}"
                            )
                        h = hp * 2 + hsel
                        nc.tensor.matmul(
                            accs[hsel][:],
                            vp_sb[:, (kc * HPC + h) * VW : (kc * HPC + h) * VW + VW],
                            e_sb[:, hsel * 512 : (hsel + 1) * 512],
                            start=(kc == 0),
                            stop=(kc == KC - 1),
                        )

                qoff = qt * 1024 + j * 512
                esq = []
                for kc in range(KC):
                    # one [128,1024] pair tile: head A scores in cols 0:512
                    # (PE rows 0-63), head B in 512:1024 (rows 64-127) —
                    # distinct row groups + banks, so the pair co-executes
                    pair = ps_mm.tile([128, 1024], F32, tag="mm", name="ps_sc")
                    for hsel in range(2):
                        p0 = hsel * 64
                        nc.tensor.matmul(
                            pair[:, hsel * 512 : (hsel + 1) * 512],
                            kt_sb[hp][p0 : p0 + 64, kc * 128 : (kc + 1) * 128],
                            qt_sb[hp][p0 : p0 + 64, qoff : qoff + 512],
                            start=True,
                            stop=True,
                        )
                    e_sb = ep.tile([128, 1024], F16, tag="e")
                    nc.scalar.activation(
                        e_sb[:], pair[:], mybir.ActivationFunctionType.Exp
                    )
                    if kc < len(pending):
                        pending[kc]()
                        if kc == len(pending) - 1:
                            pending = []
                    for u in fillers.get((si, kc), ()):
                        u()
                    # PV trails the scores/exp stream by 2 kc
                    esq.append((kc, e_sb))
                    if len(esq) > (4 if si == 0 else 2):
                        pv(*esq.pop(0))

                if si == 0:
                    # defer the last 4 PVs + v14/v15 + normalize into
                    # half-strand 1's first three iterations — HS0 is the
                    # most filler-overloaded region, HS1 has spare slack
                    def mk0(pv=pv, args=esq[0]):
                        def run():
                            pv(*args)
                        return run
                    def mk1(pv=pv, args=esq[1], steps=vs[14]):
                        def run():
                            for s in steps:
                                s()
                            pv(*args)
                        return run
                    def mk2(pv=pv, a2=esq[2], a3=esq[3], steps=vs[15],
                            accs=accs, qt=qt, hp=hp, j=j):
                        def run():
                            for s in steps:
                                s()
                            pv(*a2)
                            pv(*a3)
                            normalize(accs, qt, hp, j)
                        return run
                    pending = [mk0(), mk1(), mk2()]
                elif si < len(strands) - 1:
                    pv(*esq.pop(0))
                    def mk_pending(pv=pv, args=esq[0], accs=accs, qt=qt, hp=hp, j=j):
                        def run():
                            pv(*args)
                            normalize(accs, qt, hp, j)
                        return run
                    pending = [mk_pending()]
                else:
                    pv(*esq.pop(0))
                    # junk bridge: keep the PE busy over the final exp drain
                    # so the HAM gate doesn't re-throttle the tail
                    jt = ps_fill.tile([16, 16], F32, tag="fl", name="ps_jt")
                    for _ in range(40):
                        nc.tensor.matmul(
                            jt[:], ones_sb[:, 0:16], ones_sb[:, 0:16],
                            start=True, stop=True,
                        )
                    pv(*esq[0])
                    s4_tail[0] = True
                    dens, rs, rbs = {}, {}, {}
                    for hsel in range(2):
                        den_sb = np_.tile([1, 512], F32, tag="den", name=f"den{hsel}")
                        nc.scalar.copy(den_sb[:], accs[hsel][DK : DK + 1, :])
                        dens[hsel] = den_sb
                    for hsel in range(2):
                        r_sb = np_.tile([1, 512], F32, tag="r", name=f"r{hsel}")
                        nc.vector.reciprocal_approx_fast(r_sb[:], dens[hsel][:])
                        rs[hsel] = r_sb
                    for hsel in range(2):
                        rb_sb = np_.tile([64, 512], F32, tag="rb", name=f"rb{hsel}")
                        nc.gpsimd.partition_broadcast(rb_sb[:], rs[hsel][:])
                        rbs[hsel] = rb_sb
                    q0c = qt * 1024 + j * 512

                    def ot_mult_half(hsel, half):
                        c0 = q0c + half * 256
                        nc.vector.tensor_mul(
                            ot_sb[hp][hsel * 64 : hsel * 64 + 64, c0 : c0 + 256],
                            accs[hsel][0:DK, half * 256 : half * 256 + 256],
                            rbs[hsel][:, half * 256 : half * 256 + 256],
                        )

                    def s4_tail_unit(q, eng):
                        o_sb = op_.tile([128, D], F16, tag="o", name="o_sb")
                        ps = ps_mm.tile([128, D], F32, tag="mm", name="ps_s4t")
                        for dc2 in range(2):
                            for hp_ in range(2):
                                nc.tensor.matmul(
                                    ps[:, dc2 * 512 : (dc2 + 1) * 512],
                                    ot_sb[hp_][:, q * 128 : (q + 1) * 128],
                                    wo_sb[hp_][:, dc2 * 512 : (dc2 + 1) * 512],
                                    start=(hp_ == 0), stop=(hp_ == 1),
                                )
                        nc.scalar.copy(o_sb[:, 0:512], ps[:, 0:512])
                        nc.vector.tensor_copy(o_sb[:, 512:1024], ps[:, 512:1024])
                        for h in range(2):
                            eng.dma_start(
                                out.ap()[q * 128 : (q + 1) * 128, h * 512 : (h + 1) * 512],
                                o_sb[:, h * 512 : (h + 1) * 512],
                            )

                    ot_mult_half(0, 0)
                    ot_mult_half(1, 0)
                    s4_tail_unit(12, nc.sync)
                    s4_tail_unit(13, nc.scalar)
                    ot_mult_half(0, 1)
                    ot_mult_half(1, 1)
                    s4_tail_unit(14, nc.sync)
                    s4_tail_unit(15, nc.scalar)

    nc.compile()
    return nc
def _shard_inputs(x, W_q, W_k, W_v, W_o):
    """Build the 8 per-core input maps (fp16, C-contiguous)."""

    def pack_w(w_rows):  # [256, D] weight rows -> [128, DC*256] lhsT tiles
        wt = w_rows.T.astype(np.float16)  # [D, 256]
        return np.ascontiguousarray(
            wt.reshape(DC, 128, 256).transpose(1, 0, 2).reshape(128, DC * 256)
        )

    in_maps = []
    for c in range(NCORES):
        b, g = divmod(c, HPC)
        rows = slice(g * HPC * DK, (g + 1) * HPC * DK)
        xt = x[b].T.astype(np.float16)  # [D, S]
        xs = np.ascontiguousarray(
            xt.reshape(DC, 128, SC, 512).transpose(2, 1, 0, 3).reshape(SC, 128, DC * 512)
        )
        in_maps.append(
            {
                "xs": xs,
                "wq": pack_w(W_q[rows] * 0.125),
                "wk": pack_w(W_k[rows]),
                "wv": pack_w(W_v[rows]),
                "wo": np.ascontiguousarray(
                    W_o[:, rows].T.astype(np.float16).reshape(2, 128, D)
                ),
            }
        )
    return in_maps


def _numpy_fallback(x, attention_mask, W_q, W_k, W_v, W_o):
    """Exact reference path (only used if the mask is not all ones)."""
    out = np.empty((B, S, D), np.float32)
    for b in range(B):
        q = (x[b] @ W_q.T).reshape(S, H, DK).transpose(1, 0, 2)
        k = (x[b] @ W_k.T).reshape(S, H, DK).transpose(1, 0, 2)
        v = (x[b] @ W_v.T).reshape(S, H, DK).transpose(1, 0, 2)
        scores = np.einsum("hqd,hkd->hqk", q, k)
        scores = np.where(attention_mask[b][None, None, :] == 0, -np.inf, scores)
        scores = scores / np.sqrt(DK)
        scores -= scores.max(axis=-1, keepdims=True)
        w = np.exp(scores)
        w /= w.sum(axis=-1, keepdims=True)
        o = np.einsum("hqk,hkd->hqd", w, v).transpose(1, 0, 2).reshape(S, D)
        out[b] = o @ W_o.T
    return out


def kernel(x, attention_mask, W_q, W_k, W_v, W_o, _trace=False):
    global _CACHED_NC
    x = np.asarray(x, dtype=np.float32)
    attention_mask = np.asarray(attention_mask)
    W_q = np.asarray(W_q, dtype=np.float32)
    W_k = np.asarray(W_k, dtype=np.float32)
    W_v = np.asarray(W_v, dtype=np.float32)
    W_o = np.asarray(W_o, dtype=np.float32)

    if not np.all(attention_mask == 1):
        return _numpy_fallback(x, attention_mask, W_q, W_k, W_v, W_o)

    if _CACHED_NC is None:
        _CACHED_NC = _build_nc()
    nc = _CACHED_NC

    in_maps = _shard_inputs(x, W_q, W_k, W_v, W_o)
    res = run_bass_kernel_spmd(
        nc, in_maps, core_ids=list(range(NCORES)), trace=_trace
    )

    out = np.empty((B, S, D), np.float32)
    for b in range(B):
        acc = np.zeros((S, D), np.float64)
        for g in range(HPC):
            acc += res.results[b * HPC + g]["out"].astype(np.float64)
        out[b] = acc.astype(np.float32)
    if _trace:
        kernel.last_exec_time_ns = res.exec_time_ns
    return out


# revision 18
# speedup vs baseline: 1.0355x; 1.0013x over previous
"""Multi-head attention (B=2, S=2048, D=1024, H=16) on 8 Trainium2 NeuronCores.

Sharding: core c = (batch b = c//4) x (head-group g = c%4, 4 heads each).
Each core computes its 4 heads' attention for its batch plus the partial
output projection over its 256 W_o columns; the host sums the 4 group
partials per batch.

All matmuls fp16 (PSUM fp32).  Structure: 8 HALF-strands (qt, hp, j),
each covering 512 q columns for 2 heads.  Per kc one [128,1024] PSUM
pair-tile holds both heads' scoresT side by side (head A cols 0:512 from
PE row-group 0-63, head B cols 512:1024 from rows 64-127) so the two
score matmuls co-execute on distinct PE row tiles (Δstart ~4ns), and ONE
exp activation consumes both — the next kc's pair then unblocks as a
unit and fuses again.  Pair tiles are double-buffered (4 PSUM banks),
accs 1 bank/head ×4 (two half-strands overlap at boundaries) = 8 banks.

PV lhsT blocks are zero-padded to 128 weight columns [V(64)|ones|0*63]
so LDWEIGHTS takes the 4x fast-weight-load path; the ones column
accumulates softmax denominators in PSUM row 64.

The exp stream paces the steady state at ~1.07us/kc; PE slack per kc
(~0.3-0.4us) is filled with projection units placed just-in-time.  Each
half-strand's last two PVs + normalize defer into the next half-strand's
first iterations.  Output projection s4 units 0-7 run inside the last
two half-strands, 8-15 in the tail; output is fp16 (halves the out DMA).
"""

import sys

for _p in ("/opt/trn_rl_repo", "/root/.axon_site/_ro/trn_rl_repo"):
    if _p not in sys.path:
        sys.path.insert(0, _p)

import numpy as np

import concourse.mybir as mybir
import concourse.tile as tile
from concourse import bacc
from concourse.bass_utils import run_bass_kernel_spmd

F32 = mybir.dt.float32
F16 = mybir.dt.float16

B, S, D = 2, 2048, 1024
H, DK = 16, 64
HPC = 4          # heads per core
NCORES = 8
DC = 8           # number of 128-row chunks of D (contraction tiles)
SC = 4           # S chunks of 512 for the projections
KC = S // 128    # 16 k-chunks
VW = 128         # padded V-block width per (kc, head): [V(64) | ones | zeros]

_CACHED_NC = None


def _build_nc():
    nc = bacc.Bacc("TRN2", target_bir_lowering=False, debug=False)

    xs = nc.dram_tensor("xs", [SC, 128, DC * 512], F16, kind="ExternalInput")
    wq = nc.dram_tensor("wq", [128, DC * 256], F16, kind="ExternalInput")
    wk = nc.dram_tensor("wk", [128, DC * 256], F16, kind="ExternalInput")
    wv = nc.dram_tensor("wv", [128, DC * 256], F16, kind="ExternalInput")
    wo = nc.dram_tensor("wo", [2, 128, D], F16, kind="ExternalInput")
    out = nc.dram_tensor("out", [S, D], F16, kind="ExternalOutput")

    with tile.TileContext(nc) as tc:
        with (
            tc.tile_pool(name="persist", bufs=1) as pp,
            tc.tile_pool(name="ps_mm", bufs=2, space="PSUM") as ps_mm,
            tc.tile_pool(name="ps_acc", bufs=3, space="PSUM") as ps_acc,
            tc.tile_pool(name="ps_fill", bufs=1, space="PSUM") as ps_fill,
            tc.tile_pool(name="exp_pool", bufs=10) as ep,
            tc.tile_pool(name="out_pool", bufs=6) as op_,
            tc.tile_pool(name="nrm_pool", bufs=6) as np_,
        ):
            wk_sb = pp.tile([128, DC * 256], F16, tag="wk")
            wq_sb = pp.tile([128, DC * 256], F16, tag="wq")
            wv_sb = pp.tile([128, DC * 256], F16, tag="wv")
            x_sb = [
                pp.tile([128, DC * 512], F16, tag=f"x{i}", name=f"x_sb{i}")
                for i in range(SC)
            ]
            wo_sb = [
                pp.tile([128, D], F16, tag=f"wo{i}", name=f"wo_sb{i}")
                for i in range(2)
            ]

            # --- PE warm-up: the HAM clock gate holds the PE at 1.2GHz
            # until it sees ~3.4us of sustained busy.  Junk matmuls on the
            # ones tile (no DMA deps) warm it while the first transfers
            # land, and fill the DMA-wait gaps in the prologue chains so
            # the gate never re-throttles.
            ones_sb = pp.tile([128, 16], F32, tag="ones")
            nc.gpsimd.memset(ones_sb[:], 1.0)
            junk_ps = ps_fill.tile([16, 16], F32, tag="fl", name="ps_junk")

            def junk(n):
                for _ in range(n):
                    nc.tensor.matmul(
                        junk_ps[:], ones_sb[:, 0:16], ones_sb[:, 0:16],
                        start=True, stop=True,
                    )

            junk(80)

            # --- DMA head: chunked transfers; issue cost is ~0.6us per
            # dma_start and serializes per engine, so spread issues over the
            # three DGE-capable engines, first-needed first.
            # kt0.s0 consumes (wk.d, x0.d) pairs in d order; qt0.s0 needs wq.
            for d in range(0, DC, 2):
                nc.sync.dma_start(
                    wk_sb[:, d * 256 : (d + 1) * 256],
                    wk.ap()[:, d * 256 : (d + 1) * 256],
                )
                nc.gpsimd.dma_start(
                    x_sb[0][:, d * 512 : (d + 1) * 512],
                    xs.ap()[0][:, d * 512 : (d + 1) * 512],
                )
                nc.sync.dma_start(
                    x_sb[0][:, (d + 1) * 512 : (d + 2) * 512],
                    xs.ap()[0][:, (d + 1) * 512 : (d + 2) * 512],
                )
                nc.gpsimd.dma_start(
                    wk_sb[:, (d + 1) * 256 : (d + 2) * 256],
                    wk.ap()[:, (d + 1) * 256 : (d + 2) * 256],
                )
            for d in range(DC):
                nc.scalar.dma_start(
                    wq_sb[:, d * 256 : (d + 1) * 256],
                    wq.ap()[:, d * 256 : (d + 1) * 256],
                )
            # x1 split between sync and gpsimd (needed by kt0.s1 early in
            # half-strand 0); wv on gpsimd (v units start ~kc2)
            for h in range(2):
                nc.sync.dma_start(
                    x_sb[1][:, h * 2048 : h * 2048 + 1024],
                    xs.ap()[1][:, h * 2048 : h * 2048 + 1024],
                )
                nc.gpsimd.dma_start(
                    x_sb[1][:, h * 2048 + 1024 : (h + 1) * 2048],
                    xs.ap()[1][:, h * 2048 + 1024 : (h + 1) * 2048],
                )
            for d in range(0, DC, 2):
                nc.gpsimd.dma_start(
                    wv_sb[:, d * 256 : (d + 2) * 256],
                    wv.ap()[:, d * 256 : (d + 2) * 256],
                )
            for h in range(4):
                nc.sync.dma_start(
                    x_sb[2][:, h * 1024 : (h + 1) * 1024],
                    xs.ap()[2][:, h * 1024 : (h + 1) * 1024],
                )
                nc.scalar.dma_start(
                    x_sb[3][:, h * 1024 : (h + 1) * 1024],
                    xs.ap()[3][:, h * 1024 : (h + 1) * 1024],
                )
            for i in range(2):
                nc.scalar.dma_start(wo_sb[i][:], wo.ap()[i])

            qt_sb = [
                pp.tile([128, S], F16, tag=f"qt{i}", name=f"qt_sb{i}")
                for i in range(2)
            ]
            kt_sb = [
                pp.tile([128, S], F16, tag=f"kt{i}", name=f"kt_sb{i}")
                for i in range(2)
            ]
            vp_sb = pp.tile([128, KC * HPC * VW], F16, tag="vp")
            ot_sb = [
                pp.tile([128, S], F16, tag=f"ot{i}", name=f"ot_sb{i}")
                for i in range(2)
            ]

            # vp: per (kc, h) a 128-col block [V(64) | ones | 0*63]; zero the
            # pad + write ones via DVE in kc-order quarters so the first
            # v-copies aren't gated on the whole memset.
            for qtr in range(4):
                nc.vector.memset(vp_sb[:, qtr * 2048 : (qtr + 1) * 2048], 0.0)
                ones_ap = vp_sb[:, qtr * 2048 : (qtr + 1) * 2048].rearrange(
                    "p (b c) -> p b c", c=VW
                )[:, :, 64:65]
                nc.vector.tensor_copy(ones_ap, ones_sb[:].unsqueeze(-1))

            # ---- projection work units, split into ~0.4us STEPS so the
            # PE's in-order queue never holds a long filler chain in front
            # of the next kc's score pair (fillers own a dedicated 1-bank
            # PSUM slot and may span kc slots freely; PSUM accumulation
            # groups interleave fine across banks). ----
            def qk_steps(w_sb, t_sb, hp, sc):
                st = {}

                def mk(d0, d1, last):
                    def run():
                        if d0 == 0:
                            st["ps"] = ps_fill.tile(
                                [128, 512], F32, tag="fl", name="ps_qk"
                            )
                        for d in range(d0, d1):
                            nc.tensor.matmul(
                                st["ps"][:],
                                w_sb[:, d * 256 + hp * 128 : d * 256 + hp * 128 + 128],
                                x_sb[sc][:, d * 512 : (d + 1) * 512],
                                start=(d == 0),
                                stop=(d == DC - 1),
                            )
                        if last:
                            nc.vector.tensor_copy(
                                t_sb[hp][:, sc * 512 : (sc + 1) * 512], st["ps"][:]
                            )
                    return run

                return [mk(0, 3, False), mk(3, 6, False), mk(6, 8, True)]

            def v_steps(kc):
                sc, i = divmod(kc, 4)
                st = {}

                def mk(d0, d1, last):
                    def run():
                        if d0 == 0:
                            st["ps"] = ps_fill.tile(
                                [128, 512], F32, tag="fl", name="ps_v"
                            )
                        for d in range(d0, d1):
                            nc.tensor.matmul(
                                st["ps"][:, 0:256],
                                x_sb[sc][:, d * 512 + i * 128 : d * 512 + i * 128 + 128],
                                wv_sb[:, d * 256 : (d + 1) * 256],
                                start=(d == 0),
                                stop=(d == DC - 1),
                            )
                        if last:
                            base = kc * HPC * VW
                            dst = vp_sb[:, base : base + HPC * VW]
                            dst = dst.rearrange("p (g c) -> p g c", c=VW)[:, :, 0:DK]
                            src = st["ps"][:, 0:256].rearrange("p (g c) -> p g c", c=DK)
                            nc.vector.tensor_copy(dst, src)
                    return run

                return [mk(0, 4, False), mk(4, 8, True)]

            s4_tail = [False]

            def s4_steps(q16):
                st = {}

                def mk(dc2):
                    def run():
                        o_key = "o"
                        if dc2 == 0:
                            st[o_key] = op_.tile([128, D], F16, tag="o", name="o_sb")
                        o_sb = st[o_key]
                        ps = ps_fill.tile([128, 512], F32, tag="fl", name="ps_s4")
                        for hp in range(2):
                            nc.tensor.matmul(
                                ps[:],
                                ot_sb[hp][:, q16 * 128 : (q16 + 1) * 128],
                                wo_sb[hp][:, dc2 * 512 : (dc2 + 1) * 512],
                                start=(hp == 0),
                                stop=(hp == 1),
                            )
                        if s4_tail[0] and dc2 == 0:
                            nc.scalar.copy(
                                o_sb[:, dc2 * 512 : (dc2 + 1) * 512], ps[:]
                            )
                        else:
                            nc.vector.tensor_copy(
                                o_sb[:, dc2 * 512 : (dc2 + 1) * 512], ps[:]
                            )
                        if dc2 == 1:
                            if s4_tail[0]:
                                for h in range(2):
                                    nc.sync.dma_start(
                                        out.ap()[
                                            q16 * 128 : (q16 + 1) * 128,
                                            h * 512 : (h + 1) * 512,
                                        ],
                                        o_sb[:, h * 512 : (h + 1) * 512],
                                    )
                            else:
                                nc.sync.dma_start(
                                    out.ap()[q16 * 128 : (q16 + 1) * 128, :], o_sb[:]
                                )
                    return run

                return [mk(0), mk(1)]

            # ---- prologue: kt0.s0 and qt0.s0 interleaved per-d on the two
            # ps_mm slots (pair tiles aren't allocated yet), junk between so
            # DMA-wait gaps don't re-throttle the PE ----
            ps_kt = ps_mm.tile([128, 512], F32, tag="mm", name="ps_pkt")
            ps_qt = ps_mm.tile([128, 512], F32, tag="mm", name="ps_pqt")
            for d in range(DC):
                nc.tensor.matmul(
                    ps_kt[:],
                    wk_sb[:, d * 256 : d * 256 + 128],
                    x_sb[0][:, d * 512 : (d + 1) * 512],
                    start=(d == 0), stop=(d == DC - 1),
                )
                nc.tensor.matmul(
                    ps_qt[:],
                    wq_sb[:, d * 256 : d * 256 + 128],
                    x_sb[0][:, d * 512 : (d + 1) * 512],
                    start=(d == 0), stop=(d == DC - 1),
                )
            nc.vector.tensor_copy(kt_sb[0][:, 0:512], ps_kt[:])
            nc.vector.tensor_copy(qt_sb[0][:, 0:512], ps_qt[:])

            # ---- filler placement (si, kc) -> steps, just-in-time ----
            fillers = {}

            def place(si, kcs, steps):
                for kc, s in zip(kcs, steps):
                    fillers.setdefault((si, kc), []).append(s)

            vs = {k: v_steps(k) for k in range(KC)}
            s4s = [s4_steps(q) for q in range(16)]

            place(0, (1, 2, 3), qk_steps(wk_sb, kt_sb, 0, 1))
            place(0, (5, 6, 7), qk_steps(wk_sb, kt_sb, 0, 2))
            place(0, (9, 10, 11), qk_steps(wk_sb, kt_sb, 0, 3))
            place(0, (12, 13, 14), qk_steps(wq_sb, qt_sb, 0, 1))
            place(0, (0, 0), vs[0])
            place(0, (0, 1), vs[1])
            for k in range(2, 14):
                place(0, (k - 1, k), vs[k])
            place(1, (0, 1, 2), qk_steps(wq_sb, qt_sb, 0, 2))
            place(1, (8, 9, 10), qk_steps(wq_sb, qt_sb, 0, 3))
            place(2, (0, 1, 2), qk_steps(wk_sb, kt_sb, 1, 0))
            place(2, (6, 7, 8), qk_steps(wk_sb, kt_sb, 1, 1))
            place(2, (11, 12, 13), qk_steps(wq_sb, qt_sb, 1, 0))
            place(3, (0, 1, 2), qk_steps(wk_sb, kt_sb, 1, 2))
            place(3, (5, 6, 7), qk_steps(wk_sb, kt_sb, 1, 3))
            place(3, (10, 11, 12), qk_steps(wq_sb, qt_sb, 1, 1))
            place(4, (0, 1, 2), qk_steps(wq_sb, qt_sb, 1, 2))
            place(4, (8, 9, 10), qk_steps(wq_sb, qt_sb, 1, 3))
            place(5, (0, 1), s4s[0])
            place(5, (3, 4), s4s[1])
            place(5, (7, 8), s4s[2])
            place(5, (11, 12), s4s[3])
            place(6, (0, 1), s4s[4])
            place(6, (3, 4), s4s[5])
            place(6, (7, 8), s4s[6])
            place(6, (11, 12), s4s[7])
            place(7, (0, 1), s4s[8])
            place(7, (3, 4), s4s[9])
            place(7, (7, 8), s4s[10])
            place(7, (11, 12), s4s[11])
            # (qt, hp, j): hp-major, j innermost
            strands = [
                (0, 0, 0), (0, 0, 1), (1, 0, 0), (1, 0, 1),
                (0, 1, 0), (0, 1, 1), (1, 1, 0), (1, 1, 1),
            ]

            def normalize(accs, qt, hp, j, tail=False):
                dens, rs, rbs = {}, {}, {}
                for hsel in range(2):
                    den_sb = np_.tile([1, 512], F32, tag="den", name=f"den{hsel}")
                    if tail:
                        nc.scalar.copy(den_sb[:], accs[hsel][DK : DK + 1, :])
                    else:
                        nc.vector.tensor_copy(den_sb[:], accs[hsel][DK : DK + 1, :])
                    dens[hsel] = den_sb
                for hsel in range(2):
                    r_sb = np_.tile([1, 512], F32, tag="r", name=f"r{hsel}")
                    nc.vector.reciprocal_approx_fast(r_sb[:], dens[hsel][:])
                    rs[hsel] = r_sb
                for hsel in range(2):
                    rb_sb = np_.tile([64, 512], F32, tag="rb", name=f"rb{hsel}")
                    nc.gpsimd.partition_broadcast(rb_sb[:], rs[hsel][:])
                    rbs[hsel] = rb_sb
                q0 = qt * 1024 + j * 512
                for hsel in range(2):
                    nc.vector.tensor_mul(
                        ot_sb[hp][hsel * 64 : hsel * 64 + 64, q0 : q0 + 512],
                        accs[hsel][0:DK, :],
                        rbs[hsel][:],
                    )

            # Each half-strand's final PV + normalize defer into the NEXT
            # half-strand's first iteration (so its scores/exp launch first
            # and the exp stream never gaps at a boundary).
            pending = []

            for si, (qt, hp, j) in enumerate(strands):
                accs = {}

                def pv(kc, e_sb, accs=accs, hp=hp):
                    for hsel in range(2):
                        if hsel not in accs:
                            # lazy: the previous half-strand's accs live
                            # until its deferred normalize at our kc0
                            accs[hsel] = ps_acc.tile(
                                [128, 512], F32, tag="acc", name=f"acc{hsel}"
                            )
                        h = hp * 2 + hsel
                        nc.tensor.matmul(
                            accs[hsel][:],
                            vp_sb[:, (kc * HPC + h) * VW : (kc * HPC + h) * VW + VW],
                            e_sb[:, hsel * 512 : (hsel + 1) * 512],
                            start=(kc == 0),
                            stop=(kc == KC - 1),
                        )

                qoff = qt * 1024 + j * 512
                esq = []
                for kc in range(KC):
                    # one [128,1024] pair tile: head A scores in cols 0:512
                    # (PE rows 0-63), head B in 512:1024 (rows 64-127) —
                    # distinct row groups + banks, so the pair co-executes
                    pair = ps_mm.tile([128, 1024], F32, tag="mm", name="ps_sc")
                    for hsel in range(2):
                        p0 = hsel * 64
                        nc.tensor.matmul(
                            pair[:, hsel * 512 : (hsel + 1) * 512],
                            kt_sb[hp][p0 : p0 + 64, kc * 128 : (kc + 1) * 128],
                            qt_sb[hp][p0 : p0 + 64, qoff : qoff + 512],
                            start=True,
                            stop=True,
                        )
                    e_sb = ep.tile([128, 1024], F16, tag="e")
                    nc.scalar.activation(
                        e_sb[:], pair[:], mybir.ActivationFunctionType.Exp
                    )
                    if kc < len(pending):
                        pending[kc]()
                        if kc == len(pending) - 1:
                            pending = []
                    for u in fillers.get((si, kc), ()):
                        u()
                    # PV trails the scores/exp stream by 2 kc
                    esq.append((kc, e_sb))
                    if len(esq) > (4 if si == 0 else 2):
                        pv(*esq.pop(0))

                if si == 0:
                    # defer the last 4 PVs + v14/v15 + normalize into
                    # half-strand 1's first three iterations — HS0 is the
                    # most filler-overloaded region, HS1 has spare slack
                    def mk0(pv=pv, args=esq[0]):
                        def run():
                            pv(*args)
                        return run
                    def mk1(pv=pv, args=esq[1], steps=vs[14]):
                        def run():
                            for s in steps:
                                s()
                            pv(*args)
                        return run
                    def mk2(pv=pv, a2=esq[2], a3=esq[3], steps=vs[15],
                            accs=accs, qt=qt, hp=hp, j=j):
                        def run():
                            for s in steps:
                                s()
                            pv(*a2)
                            pv(*a3)
                            normalize(accs, qt, hp, j)
                        return run
                    pending = [mk0(), mk1(), mk2()]
                elif si < len(strands) - 1:
                    pv(*esq.pop(0))
                    def mk_pending(pv=pv, args=esq[0], accs=accs, qt=qt, hp=hp, j=j):
                        def run():
                            pv(*args)
                            normalize(accs, qt, hp, j)
                        return run
                    pending = [mk_pending()]
                else:
                    pv(*esq.pop(0))
                    # junk bridge: keep the PE busy over the final exp drain
                    # so the HAM gate doesn't re-throttle the tail
                    jt = ps_fill.tile([16, 16], F32, tag="fl", name="ps_jt")
                    for _ in range(40):
                        nc.tensor.matmul(
                            jt[:], ones_sb[:, 0:16], ones_sb[:, 0:16],
                            start=True, stop=True,
                        )
                    pv(*esq[0])
                    s4_tail[0] = True
                    dens, rs, rbs = {}, {}, {}
                    for hsel in range(2):
                        den_sb = np_.tile([1, 512], F32, tag="den", name=f"den{hsel}")
                        nc.scalar.copy(den_sb[:], accs[hsel][DK : DK + 1, :])
                        dens[hsel] = den_sb
                    for hsel in range(2):
                        r_sb = np_.tile([1, 512], F32, tag="r", name=f"r{hsel}")
                        nc.vector.reciprocal_approx_fast(r_sb[:], dens[hsel][:])
                        rs[hsel] = r_sb
                    for hsel in range(2):
                        rb_sb = np_.tile([64, 512], F32, tag="rb", name=f"rb{hsel}")
                        nc.gpsimd.partition_broadcast(rb_sb[:], rs[hsel][:])
                        rbs[hsel] = rb_sb
                    q0c = qt * 1024 + j * 512

                    def ot_mult_half(hsel, half):
                        c0 = q0c + half * 256
                        nc.vector.tensor_mul(
                            ot_sb[hp][hsel * 64 : hsel * 64 + 64, c0 : c0 + 256],
                            accs[hsel][0:DK, half * 256 : half * 256 + 256],
                            rbs[hsel][:, half * 256 : half * 256 + 256],
                        )

                    def s4_tail_unit(q, eng):
                        o_sb = op_.tile([128, D], F16, tag="o", name="o_sb")
                        ps = ps_mm.tile([128, D], F32, tag="mm", name="ps_s4t")
                        for dc2 in range(2):
                            for hp_ in range(2):
                                nc.tensor.matmul(
                                    ps[:, dc2 * 512 : (dc2 + 1) * 512],
                                    ot_sb[hp_][:, q * 128 : (q + 1) * 128],
                                    wo_sb[hp_][:, dc2 * 512 : (dc2 + 1) * 512],
                                    start=(hp_ == 0), stop=(hp_ == 1),
                                )
                        nc.scalar.copy(o_sb[:, 0:512], ps[:, 0:512])
                        nc.vector.tensor_copy(o_sb[:, 512:1024], ps[:, 512:1024])
                        for h in range(2):
                            eng.dma_start(
                                out.ap()[q * 128 : (q + 1) * 128, h * 512 : (h + 1) * 512],
                                o_sb[:, h * 512 : (h + 1) * 512],
                            )

                    ot_mult_half(0, 0)
                    ot_mult_half(1, 0)
                    s4_tail_unit(12, nc.sync)
                    s4_tail_unit(13, nc.scalar)
                    ot_mult_half(0, 1)
                    ot_mult_half(1, 1)
                    s4_tail_unit(14, nc.sync)
                    s4_tail_unit(15, nc.scalar)

    nc.compile()
    return nc
def _shard_inputs(x, W_q, W_k, W_v, W_o):
    """Build the 8 per-core input maps (fp16, C-contiguous)."""

    def pack_w(w_rows):  # [256, D] weight rows -> [128, DC*256] lhsT tiles
        wt = w_rows.T.astype(np.float16)  # [D, 256]
        return np.ascontiguousarray(
            wt.reshape(DC, 128, 256).transpose(1, 0, 2).reshape(128, DC * 256)
        )

    in_maps = []
    for c in range(NCORES):
        b, g = divmod(c, HPC)
        rows = slice(g * HPC * DK, (g + 1) * HPC * DK)
        xt = x[b].T.astype(np.float16)  # [D, S]
        xs = np.ascontiguousarray(
            xt.reshape(DC, 128, SC, 512).transpose(2, 1, 0, 3).reshape(SC, 128, DC * 512)
        )
        in_maps.append(
            {
                "xs": xs,
                "wq": pack_w(W_q[rows] * 0.125),
                "wk": pack_w(W_k[rows]),
                "wv": pack_w(W_v[rows]),
                "wo": np.ascontiguousarray(
                    W_o[:, rows].T.astype(np.float16).reshape(2, 128, D)
                ),
            }
        )
    return in_maps


def _numpy_fallback(x, attention_mask, W_q, W_k, W_v, W_o):
    """Exact reference path (only used if the mask is not all ones)."""
    out = np.empty((B, S, D), np.float32)
    for b in range(B):
        q = (x[b] @ W_q.T).reshape(S, H, DK).transpose(1, 0, 2)
        k = (x[b] @ W_k.T).reshape(S, H, DK).transpose(1, 0, 2)
        v = (x[b] @ W_v.T).reshape(S, H, DK).transpose(1, 0, 2)
        scores = np.einsum("hqd,hkd->hqk", q, k)
        scores = np.where(attention_mask[b][None, None, :] == 0, -np.inf, scores)
        scores = scores / np.sqrt(DK)
        scores -= scores.max(axis=-1, keepdims=True)
        w = np.exp(scores)
        w /= w.sum(axis=-1, keepdims=True)
        o = np.einsum("hqk,hkd->hqd", w, v).transpose(1, 0, 2).reshape(S, D)
        out[b] = o @ W_o.T
    return out


def kernel(x, attention_mask, W_q, W_k, W_v, W_o, _trace=False):
    global _CACHED_NC
    x = np.asarray(x, dtype=np.float32)
    attention_mask = np.asarray(attention_mask)
    W_q = np.asarray(W_q, dtype=np.float32)
    W_k = np.asarray(W_k, dtype=np.float32)
    W_v = np.asarray(W_v, dtype=np.float32)
    W_o = np.asarray(W_o, dtype=np.float32)

    if not np.all(attention_mask == 1):
        return _numpy_fallback(x, attention_mask, W_q, W_k, W_v, W_o)

    if _CACHED_NC is None:
        _CACHED_NC = _build_nc()
    nc = _CACHED_NC

    in_maps = _shard_inputs(x, W_q, W_k, W_v, W_o)
    res = run_bass_kernel_spmd(
        nc, in_maps, core_ids=list(range(NCORES)), trace=_trace
    )

    out = np.empty((B, S, D), np.float32)
    for b in range(B):
        acc = np.zeros((S, D), np.float64)
        for g in range(HPC):
            acc += res.results[b * HPC + g]["out"].astype(np.float64)
        out[b] = acc.astype(np.float32)
    if _trace:
        kernel.last_exec_time_ns = res.exec_time_ns
    return out
